# revision 1
# baseline (speedup 1.0000x reference)
"""GATv2 message-passing + dueling Q head on 8 Trainium2 NeuronCores.

Sharding: nodes (and incident edges, cut by destination) are split into 8
contiguous ranges of 6250 nodes, one per core; graph boundaries align with
core boundaries so pooling and the MLP head run fully per-core. Each core
computes the full xl/xr linear transforms (replicated, f32r matmuls), then
processes its own edges: xl[src] rows are fetched with indirect DMA, xr[dst]
is expanded on-chip from a per-block contiguous load via a PE matmul against
the transposed edge->slot selection matrix. Segment softmax is computed
without max-subtraction (scores are O(1), exp is safe) and the weighted
scatter-add is a PE matmul with the 0/1 selection matrix, accumulated in
PSUM per 128-node block.
"""
import sys
import math
import time
import numpy as np

_REPO = "/opt/trn_rl_repo"

N = 50000
E = 800000
G = 64
HC = 128
H = 4
C = 32
ACT_DIM = 10
MLP_H = 128
NEG = 0.2
NCORES = 8
NPC = N // NCORES            # 6250 nodes per core
P = 128
NBLK = math.ceil(NPC / P)    # 49 blocks of <=128 dst nodes
NPAD = 392 * P               # 50176 padded node count
EGRP = 4                     # edge tiles per macro tile

_timing = {}


def _host_prep(inputs):
    ei = inputs["edge_index"].astype(np.int64)
    src = np.concatenate([ei[0], np.arange(N, dtype=np.int64)])
    dst = np.concatenate([ei[1], np.arange(N, dtype=np.int64)])
    core = dst // NPC

    per_core = []
    max_tiles = 1
    for k in range(NCORES):
        m = core == k
        s_k = src[m]
        d_k = dst[m] - k * NPC
        order = np.argsort(d_k, kind="stable")
        s_k = s_k[order]
        d_k = d_k[order]
        blk = d_k // P
        cnt = np.bincount(blk, minlength=NBLK)
        max_tiles = max(max_tiles, int(np.ceil(cnt.max() / P)))
        per_core.append((s_k, d_k, blk, cnt))

    t_b = int(math.ceil(max_tiles / EGRP) * EGRP)   # tiles per block, EGRP-aligned
    nt = NBLK * t_b

    esrc_all, eslot_all, nodeid_all = [], [], []
    for k in range(NCORES):
        s_k, d_k, blk, cnt = per_core[k]
        esrc = np.zeros((nt, P), np.int32)
        eslot = np.full((nt, P), -1.0, np.float32)
        pos = 0
        for b in range(NBLK):
            nb = int(cnt[b])
            rows = s_k[pos:pos + nb]
            slots = (d_k[pos:pos + nb] - b * P).astype(np.float32)
            pos += nb
            t0 = b * t_b
            flat_src = esrc[t0:t0 + t_b].reshape(-1)
            flat_slot = eslot[t0:t0 + t_b].reshape(-1)
            flat_src[:nb] = rows
            flat_slot[:nb] = slots
        esrc_all.append(np.ascontiguousarray(esrc.T))       # [P, nt]
        eslot_all.append(np.ascontiguousarray(eslot.T))     # [P, nt]
        nid = np.zeros((NBLK, P), np.int32)
        base = k * NPC
        for b in range(NBLK):
            ids = np.arange(b * P, min((b + 1) * P, NPC)) + base
            nid[b, :len(ids)] = ids
            nid[b, len(ids):] = base
        nodeid_all.append(np.ascontiguousarray(nid.T))      # [P, NBLK]

    # graph chunk structure (identical on every core)
    lb = [int(math.ceil(N * j / G)) for j in range(9)]     # local graph bounds
    chunks = []   # (block, graph_j, lo, hi) node-local within block
    for b in range(NBLK):
        blo, bhi = b * P, min((b + 1) * P, NPC)
        for j in range(8):
            lo, hi = max(lb[j], blo), min(lb[j + 1], bhi)
            if lo < hi:
                chunks.append((b, j, lo - blo, hi - blo))
    return t_b, nt, esrc_all, eslot_all, nodeid_all, chunks


def _build(inputs, t_b, nt, chunks):
    if _REPO not in sys.path:
        sys.path.insert(0, _REPO)
    from contextlib import ExitStack
    import concourse.bass as bass
    import concourse.bacc as bacc
    import concourse.tile as tile
    from concourse import mybir

    f32 = mybir.dt.float32
    f32r = mybir.dt.float32r
    i32 = mybir.dt.int32
    AL = mybir.AluOpType
    AF = mybir.ActivationFunctionType

    nc = bacc.Bacc("TRN2", target_bir_lowering=False, debug=False,
                   enable_asserts=False, num_devices=NCORES)

    def din(name, shape, dt):
        return nc.dram_tensor(name, shape, dt, kind="ExternalInput").ap()

    xT = din("xT", [P, NPAD], f32r)
    wlwr = din("wlwr", [P, 2 * HC], f32r)
    blbr = din("blbr", [1, 2 * HC], f32r)
    ones1r = din("ones1r", [1, P], f32r)
    esrcT = din("esrcT", [P, nt], i32)
    eslotT = din("eslotT", [P, nt], f32)
    nodeidT = din("nodeidT", [P, NBLK], i32)
    iota_c = din("iota_c", [P, P], f32)
    ident_c = din("ident_c", [P, P], f32)
    att4_c = din("att4_c", [P, EGRP * HC], f32)
    cb_c = din("cb_c", [P, P], f32)
    wq1_c = din("wq1_c", [HC, MLP_H], f32)
    wq2_c = din("wq2_c", [MLP_H, ACT_DIM], f32)
    wv1_c = din("wv1_c", [HC, MLP_H], f32)
    wv2_c = din("wv2_c", [MLP_H, 1], f32)
    wq2nm_c = din("wq2nm_c", [MLP_H, 1], f32)
    bq1_c = din("bq1_c", [MLP_H, 1], f32)
    bv1_c = din("bv1_c", [MLP_H, 1], f32)
    bq2_c = din("bq2_c", [ACT_DIM, 1], f32)
    ones110 = din("ones110", [1, ACT_DIM], f32)
    cadd = float(inputs["bv2"][0] - inputs["bq2"].sum() / ACT_DIM)

    xl_d = nc.dram_tensor("xl_d", [NPAD, HC], f32, kind="Internal").ap()
    xr_d = nc.dram_tensor("xr_d", [NPAD, HC], f32, kind="Internal").ap()
    out_q = nc.dram_tensor("out_q", [ACT_DIM, 8], f32, kind="ExternalOutput").ap()

    blk_chunks = {}
    for (b, j, lo, hi) in chunks:
        blk_chunks.setdefault(b, []).append((j, lo, hi))

    with tile.TileContext(nc) as tc, ExitStack() as ctx:
        cp = ctx.enter_context(tc.tile_pool(name="consts", bufs=1))

        def cload(name, ap_in, shape, dt):
            t = cp.tile(shape, dt, tag=name)
            nc.sync.dma_start(t[:], ap_in)
            return t

        iota_t = cload("iota", iota_c[:], [P, P], f32)
        ident_t = cload("ident", ident_c[:], [P, P], f32)
        att4_t = cload("att4", att4_c[:], [P, EGRP * HC], f32)
        cb_t = cload("cb", cb_c[:], [P, P], f32)
        wlwr_t = cload("wlwr", wlwr[:], [P, 2 * HC], f32r)
        blbr_t = cload("blbr", blbr[:], [1, 2 * HC], f32r)
        ones1r_t = cload("ones1r", ones1r[:], [1, P], f32r)
        esrc_t = cload("esrc", esrcT[:], [P, nt], i32)
        eslot_t = cload("eslot", eslotT[:], [P, nt], f32)
        nid_t = cload("nid", nodeidT[:], [P, NBLK], i32)
        wq1_t = cload("wq1", wq1_c[:], [HC, MLP_H], f32)
        wq2_t = cload("wq2", wq2_c[:], [MLP_H, ACT_DIM], f32)
        wv1_t = cload("wv1", wv1_c[:], [HC, MLP_H], f32)
        wv2_t = cload("wv2", wv2_c[:], [MLP_H, 1], f32)
        wq2nm_t = cload("wq2nm", wq2nm_c[:], [MLP_H, 1], f32)
        bq1_t = cload("bq1", bq1_c[:], [MLP_H, 1], f32)
        bv1_t = cload("bv1", bv1_c[:], [MLP_H, 1], f32)
        bq2_t = cload("bq2", bq2_c[:], [ACT_DIM, 1], f32)
        ones110_t = cload("ones110", ones110[:], [1, ACT_DIM], f32)

        # ---------------- phase 1: xl / xr for all nodes ----------------
        p1l = ctx.enter_context(tc.tile_pool(name="p1l", bufs=8))
        p1o = ctx.enter_context(tc.tile_pool(name="p1o", bufs=6))
        p1p_cm = tc.tile_pool(name="p1p", bufs=2, space="PSUM")
        p1p = p1p_cm.__enter__()
        for j in range(NPAD // P):
            lt = p1l.tile([P, P], f32r, tag="xt")
            nc.sync.dma_start(lt[:], xT[:, j * P:(j + 1) * P])
            pt = p1p.tile([P, 2 * HC], f32, tag="p1ps")
            nc.tensor.matmul(pt[:], ones1r_t[:], blbr_t[:], start=True, stop=False)
            nc.tensor.matmul(pt[:], lt[:], wlwr_t[:], start=False, stop=True)
            ot = p1o.tile([P, 2 * HC], f32, tag="p1out")
            nc.scalar.activation(ot[:], pt[:], AF.Copy)
            nc.sync.dma_start(xl_d[j * P:(j + 1) * P, :], ot[:, 0:HC])
            nc.sync.dma_start(xr_d[j * P:(j + 1) * P, :], ot[:, HC:2 * HC])

        p1p_cm.__exit__(None, None, None)

        # ---------------- phase 2: edges ----------------
        gx = ctx.enter_context(tc.tile_pool(name="gx", bufs=6))
        wk = ctx.enter_context(tc.tile_pool(name="wk", bufs=3))
        xrp = ctx.enter_context(tc.tile_pool(name="xrp", bufs=3))
        fl = ctx.enter_context(tc.tile_pool(name="fl", bufs=2))
        pps_cm = tc.tile_pool(name="pps", bufs=1, space="PSUM")
        pps = pps_cm.__enter__()
        agg_cm = tc.tile_pool(name="agg", bufs=2, space="PSUM")
        agg = agg_cm.__enter__()
        flp_cm = tc.tile_pool(name="flp", bufs=1, space="PSUM")
        flp = flp_cm.__enter__()

        gtmp = cp.tile([P, 8, 8], f32, tag="gtmp")
        nc.gpsimd.memset(gtmp[:], -3.0e38)
        chunk_ctr = [0] * 8

        n_macro = t_b // EGRP
        for b in range(NBLK):
            xr_blk = xrp.tile([P, HC], f32, tag="xrblk")
            nc.gpsimd.indirect_dma_start(
                out=xr_blk[:], out_offset=None, in_=xr_d[:],
                in_offset=bass.IndirectOffsetOnAxis(ap=nid_t[:, b:b + 1], axis=0))
            ps_agg = agg.tile([P, HC + H], f32, tag="psagg")
            for mm in range(n_macro):
                t0 = b * t_b + mm * EGRP
                xlm = gx.tile([P, EGRP * HC], f32, tag="xlm")
                for g in range(EGRP):
                    nc.gpsimd.indirect_dma_start(
                        out=xlm[:, g * HC:(g + 1) * HC], out_offset=None,
                        in_=xl_d[:],
                        in_offset=bass.IndirectOffsetOnAxis(
                            ap=esrc_t[:, t0 + g:t0 + g + 1], axis=0))
                selm = wk.tile([P, EGRP * P], f32, tag="selm")
                for g in range(EGRP):
                    nc.vector.tensor_scalar(
                        selm[:, g * P:(g + 1) * P], iota_t[:],
                        eslot_t[:, t0 + g:t0 + g + 1], None, AL.is_equal)
                selT_ps = pps.tile([P, EGRP * P], f32, tag="selT")
                for g in range(EGRP):
                    nc.tensor.transpose(selT_ps[:, g * P:(g + 1) * P],
                                        selm[:, g * P:(g + 1) * P], ident_t[:])
                selT = wk.tile([P, EGRP * P], f32, tag="selTs")
                nc.scalar.activation(selT[:], selT_ps[:], AF.Copy)
                xr_ps = pps.tile([P, EGRP * HC], f32, tag="xre")
                for g in range(EGRP):
                    nc.tensor.matmul(xr_ps[:, g * HC:(g + 1) * HC],
                                     selT[:, g * P:(g + 1) * P], xr_blk[:],
                                     start=True, stop=True)
                sm = wk.tile([P, EGRP * HC], f32, tag="sm")
                nc.vector.tensor_tensor(sm[:], xlm[:], xr_ps[:], op=AL.add)
                tm = wk.tile([P, EGRP * HC], f32, tag="tm")
                nc.scalar.activation(tm[:], sm[:], AF.Prelu, alpha=NEG)
                um = wk.tile([P, EGRP * HC], f32, tag="um")
                nc.vector.tensor_tensor(um[:], tm[:], att4_t[:], op=AL.mult)
                em = wk.tile([P, EGRP * H], f32, tag="em")
                nc.vector.tensor_reduce(
                    em[:], um[:].rearrange("p (q c) -> p q c", c=C),
                    axis=mybir.AxisListType.X, op=AL.add)
                wm = wk.tile([P, EGRP * H], f32, tag="wm")
                nc.scalar.activation(wm[:], em[:], AF.Exp)
                msgw = wk.tile([P, EGRP, HC + H], f32, tag="msgw")
                nc.vector.tensor_tensor(
                    msgw[:, :, 0:HC].rearrange("p g (h c) -> p g h c", h=H),
                    xlm[:].rearrange("p (g h c) -> p g h c", g=EGRP, h=H),
                    wm[:].rearrange("p (g h) -> p g h", g=EGRP)
                        .to_broadcast([P, EGRP, H, C]),
                    op=AL.mult)
                nc.vector.tensor_copy(
                    msgw[:, :, HC:HC + H],
                    wm[:].rearrange("p (g h) -> p g h", g=EGRP))
                for g in range(EGRP):
                    ti = mm * EGRP + g
                    nc.tensor.matmul(ps_agg[:], selm[:, g * P:(g + 1) * P],
                                     msgw[:, g, :], start=(ti == 0),
                                     stop=(ti == t_b - 1))
            # ---- flush block b ----
            rcp = fl.tile([P, H], f32, tag="rcp")
            nc.vector.reciprocal(rcp[:], ps_agg[:, HC:HC + H])
            outb = fl.tile([P, HC], f32, tag="outb")
            nc.vector.tensor_tensor(
                outb[:].rearrange("p (h c) -> p h c", h=H),
                ps_agg[:, 0:HC].rearrange("p (h c) -> p h c", h=H),
                rcp[:].to_broadcast([P, H, C]), op=AL.mult)
            outc = fl.tile([P, HC], f32, tag="outc")
            nc.vector.tensor_tensor(outc[:], outb[:], cb_t[:], op=AL.add)
            tp_ps = flp.tile([P, P], f32, tag="tpps")
            nc.tensor.transpose(tp_ps[:], outc[:], ident_t[:])
            for (j, lo, hi) in blk_chunks.get(b, []):
                ci = chunk_ctr[j]
                chunk_ctr[j] += 1
                nc.vector.tensor_reduce(
                    gtmp[:, j, ci:ci + 1],
                    tp_ps[:, lo:hi], axis=mybir.AxisListType.X, op=AL.max)

        flp_cm.__exit__(None, None, None)
        agg_cm.__exit__(None, None, None)
        pps_cm.__exit__(None, None, None)

        # ---------------- pooling + dueling head ----------------
        gacc = fl.tile([P, 8], f32, tag="gacc")
        nc.vector.tensor_reduce(gacc[:], gtmp[:], axis=mybir.AxisListType.X,
                                op=AL.max)
        grelu = fl.tile([P, 8], f32, tag="grelu")
        nc.scalar.activation(grelu[:], gacc[:], AF.Relu)

        mp_cm = tc.tile_pool(name="mlp", bufs=1, space="PSUM")
        mp = mp_cm.__enter__()
        q1p = mp.tile([MLP_H, 8], f32, tag="q1p")
        nc.tensor.matmul(q1p[:], wq1_t[:], grelu[:], start=True, stop=True)
        q1s = fl.tile([MLP_H, 8], f32, tag="q1s")
        nc.scalar.activation(q1s[:], q1p[:], AF.Relu, bias=bq1_t[:, 0:1])
        v1p = mp.tile([MLP_H, 8], f32, tag="v1p")
        nc.tensor.matmul(v1p[:], wv1_t[:], grelu[:], start=True, stop=True)
        v1s = fl.tile([MLP_H, 8], f32, tag="v1s")
        nc.scalar.activation(v1s[:], v1p[:], AF.Relu, bias=bv1_t[:, 0:1])

        cvp = mp.tile([1, 8], f32, tag="cvp")
        nc.tensor.matmul(cvp[:], wv2_t[:], v1s[:], start=True, stop=False)
        nc.tensor.matmul(cvp[:], wq2nm_t[:], q1s[:], start=False, stop=True)
        corr = fl.tile([1, 8], f32, tag="corr")
        nc.vector.tensor_scalar(corr[:], cvp[:], cadd, None, AL.add)

        q2p = mp.tile([ACT_DIM, 8], f32, tag="q2p")
        nc.tensor.matmul(q2p[:], wq2_t[:], q1s[:], start=True, stop=False)
        nc.tensor.matmul(q2p[:], ones110_t[:], corr[:], start=False, stop=True)
        outsb = fl.tile([ACT_DIM, 8], f32, tag="outsb")
        nc.vector.tensor_scalar(outsb[:], q2p[:], bq2_t[:, 0:1], None, AL.add)
        nc.sync.dma_start(out_q[:], outsb[:])
        mp_cm.__exit__(None, None, None)

    nc.compile()
    return nc


def kernel(**inputs):
    if _REPO not in sys.path:
        sys.path.insert(0, _REPO)
    from concourse.bass_utils import run_bass_kernel_spmd

    batch = inputs["batch"]
    assert np.array_equal(batch, ((np.arange(N) * G) // N).astype(batch.dtype))

    t_b, nt, esrc_all, eslot_all, nodeid_all, chunks = _host_prep(inputs)
    nc = _build(inputs, t_b, nt, chunks)

    x = np.asarray(inputs["x"], np.float32)
    xTp = np.zeros((P, NPAD), np.float32)
    xTp[:, :N] = x.T
    att_flat = np.asarray(inputs["att"], np.float32).reshape(-1)
    shared = dict(
        xT=np.ascontiguousarray(xTp),
        wlwr=np.ascontiguousarray(
            np.concatenate([inputs["Wl"], inputs["Wr"]], axis=1).astype(np.float32)),
        blbr=np.concatenate([inputs["bl"], inputs["br"]]).astype(np.float32)[None, :],
        ones1r=np.ones((1, P), np.float32),
        iota_c=np.tile(np.arange(P, dtype=np.float32), (P, 1)),
        ident_c=np.eye(P, dtype=np.float32),
        att4_c=np.tile(att_flat, (P, EGRP)),
        cb_c=np.tile(inputs["conv_bias"].astype(np.float32), (P, 1)),
        wq1_c=np.asarray(inputs["Wq1"], np.float32),
        wq2_c=np.asarray(inputs["Wq2"], np.float32),
        wv1_c=np.asarray(inputs["Wv1"], np.float32),
        wv2_c=np.asarray(inputs["Wv2"], np.float32),
        wq2nm_c=(-np.asarray(inputs["Wq2"], np.float32).sum(1) / ACT_DIM)[:, None],
        bq1_c=np.asarray(inputs["bq1"], np.float32)[:, None],
        bv1_c=np.asarray(inputs["bv1"], np.float32)[:, None],
        bq2_c=np.asarray(inputs["bq2"], np.float32)[:, None],
        ones110=np.ones((1, ACT_DIM), np.float32),
    )
    in_maps = []
    for k in range(NCORES):
        m = dict(shared)
        m["esrcT"] = esrc_all[k]
        m["eslotT"] = eslot_all[k]
        m["nodeidT"] = nodeid_all[k]
        in_maps.append(m)

    t0 = time.time()
    res = run_bass_kernel_spmd(nc, in_maps, core_ids=list(range(NCORES)))
    _timing["first_run_s"] = time.time() - t0
    t0 = time.time()
    res = run_bass_kernel_spmd(nc, in_maps, core_ids=list(range(NCORES)))
    _timing["second_run_s"] = time.time() - t0

    out = np.concatenate([res.results[k]["out_q"].T for k in range(NCORES)], axis=0)
    return out.astype(np.float32)



# revision 11
# speedup vs baseline: 167.6884x; 167.6884x over previous
"""GATv2 message-passing + dueling Q head on 8 Trainium2 NeuronCores.

Sharding: nodes (and incident edges, cut by destination) split into 8
contiguous ranges of 6250 nodes; graph boundaries align with core boundaries
so pooling and the MLP head run per-core. Each core computes xl = x@Wl for
ALL nodes in bf16 (written to DRAM) and xr = x@Wr for its OWN nodes (kept
resident in SBUF). Edges are grouped by destination block (128 dst slots);
per-edge xl rows are fetched with dma_gather (4 parallel SWDGE queues;
int16 indices force a lo/hi split of the node table), per-edge xr rows are
expanded on-chip via a PE matmul against the transposed slot-selection
matrix, and xl is accumulated into the same PSUM via an identity matmul.
Segment softmax runs without max-subtraction (scores are O(0.1)); weighted
messages and softmax denominators are scatter-added per block with one PE
matmul per edge tile.
"""
import sys
import math
import time
import numpy as np

_REPO = "/opt/trn_rl_repo"

N = 50000
E = 800000
G = 64
HC = 128
H = 4
C = 32
ACT_DIM = 10
MLP_H = 128
NEG = 0.2
NCORES = 8
NPC = N // NCORES            # 6250 nodes per core
P = 128
NBLK = math.ceil(NPC / P)    # 49 blocks of <=128 dst nodes
NPAD = 392 * P               # 50176 padded node count
LO = 32768                   # int16-addressable rows of xl_d
XRB = 52                     # own-node blocks padded to multiple of 4
EGRP = 4                     # edge tiles per macro
CALL_T = 8                   # tiles per dma_gather call (<=1024 idxs)

_timing = {}
_cached = {}


def rerun():
    """Re-run the last compiled kernel (for profiling from test.py)."""
    from concourse.bass_utils import run_bass_kernel_spmd
    return run_bass_kernel_spmd(_cached["nc"], _cached["in_maps"],
                                core_ids=list(range(NCORES)))


def _host_prep(inputs):
    ei = inputs["edge_index"].astype(np.int64)
    src = np.concatenate([ei[0], np.arange(N, dtype=np.int64)])
    dst = np.concatenate([ei[1], np.arange(N, dtype=np.int64)])
    core = dst // NPC

    # per (core, block): lo/hi edge lists (src, slot)
    per = [[None] * NBLK for _ in range(NCORES)]
    for k in range(NCORES):
        m = core == k
        s_k = src[m]
        d_k = dst[m] - k * NPC
        blk = d_k // P
        slot = d_k % P
        for b in range(NBLK):
            mb = blk == b
            sb = s_k[mb]
            sl = slot[mb]
            lo_m = sb < LO
            per[k][b] = ((sb[lo_m], sl[lo_m]), (sb[~lo_m] - LO, sl[~lo_m]))

    t_lo = [0] * NBLK
    t_hi = [0] * NBLK
    for b in range(NBLK):
        for k in range(NCORES):
            (slo, _), (shi, _) = per[k][b]
            t_lo[b] = max(t_lo[b], (len(slo) + P - 1) // P, 1)
            t_hi[b] = max(t_hi[b], (len(shi) + P - 1) // P, 1)

    # compile-time call list + tile layout (shared across cores)
    cum_t = [0] * (NBLK + 1)
    for b in range(NBLK):
        cum_t[b + 1] = cum_t[b] + t_lo[b] + t_hi[b]
    nt_tot = cum_t[NBLK]

    calls = []  # (half, idx_col_off, block, tile_off_in_block, ntiles)
    idx_cols = 0
    for b in range(NBLK):
        for half, tcnt, base in ((0, t_lo[b], 0), (1, t_hi[b], t_lo[b])):
            done = 0
            while done < tcnt:
                ct = min(CALL_T, tcnt - done)
                calls.append((half, idx_cols, b, base + done, ct))
                idx_cols += ct * P // 16
                done += ct

    # per-core tables
    idx_all, eslot_all, xrsrc_all = [], [], []
    xT = np.zeros((P, NPAD), np.float32)
    xT[:, :N] = np.asarray(inputs["x"], np.float32).T
    for k in range(NCORES):
        idx_flat = np.zeros(idx_cols * 16, np.int16)
        eslot = np.full((P, nt_tot), -1.0, np.float32)
        for b in range(NBLK):
            for half, tcnt, base in ((0, t_lo[b], 0), (1, t_hi[b], t_lo[b])):
                sb, sl = per[k][b][half]
                ne = len(sb)
                tile0 = cum_t[b] + base
                lanes = np.arange(ne)
                eslot[lanes % P, tile0 + lanes // P] = sl
                # write indices in tile order into the call regions
                # call entries for this (b, half) run start at the call list
                # offsets recorded above
                pos = 0
                for (h2, coff, b2, toff, ct) in calls:
                    if b2 != b or h2 != half or toff < base or toff >= base + tcnt:
                        continue
                    n_here = min(ne - pos, ct * P)
                    if n_here > 0:
                        idx_flat[coff * 16: coff * 16 + n_here] = sb[pos:pos + n_here]
                        pos += n_here
        # pack: entry i of each call region -> partition i%16, col i//16 (x8)
        packed = np.zeros((P, idx_cols), np.int16)
        for (h2, coff, b2, toff, ct) in calls:
            n = ct * P
            w = idx_flat[coff * 16: coff * 16 + n].reshape(n // 16, 16).T
            packed[:, coff: coff + n // 16] = np.tile(w, (8, 1))
        idx_all.append(packed)
        eslot_all.append(eslot)

        xs = np.zeros((P, XRB * P), np.float32)
        hi = min(N - k * NPC, XRB * P)
        xs[:, :hi] = xT[:, k * NPC: k * NPC + hi]
        xrsrc_all.append(xs)

    # graph chunk structure (identical on every core)
    lb = [int(math.ceil(N * j / G)) for j in range(9)]
    chunks = []   # (block, graph_j, lo, hi) node-local within block
    for b in range(NBLK):
        blo, bhi = b * P, min((b + 1) * P, NPC)
        for j in range(8):
            lo_, hi_ = max(lb[j], blo), min(lb[j + 1], bhi)
            if lo_ < hi_:
                chunks.append((b, j, lo_ - blo, hi_ - blo))
    return (t_lo, t_hi, cum_t, nt_tot, calls, idx_cols,
            idx_all, eslot_all, xrsrc_all, xT, chunks)


def _bf16(x):
    import ml_dtypes
    return np.asarray(x, np.float32).astype(ml_dtypes.bfloat16)


def _build(inputs, prep):
    if _REPO not in sys.path:
        sys.path.insert(0, _REPO)
    from contextlib import ExitStack
    import concourse.bass as bass
    import concourse.bacc as bacc
    import concourse.tile as tile
    from concourse import mybir

    (t_lo, t_hi, cum_t, nt_tot, calls, idx_cols,
     idx_all, eslot_all, xrsrc_all, xT, chunks) = prep

    f32 = mybir.dt.float32
    bf16 = mybir.dt.bfloat16
    i16 = mybir.dt.int16
    AL = mybir.AluOpType
    AF = mybir.ActivationFunctionType

    nc = bacc.Bacc("TRN2", target_bir_lowering=False, debug=False,
                   enable_asserts=False, num_devices=NCORES,
                   num_swdge_queues=4)

    def din(name, shape, dt):
        return nc.dram_tensor(name, shape, dt, kind="ExternalInput").ap()

    xT_d = din("xT", [P, NPAD], bf16)
    xrsrc_d = din("xrsrc", [P, XRB * P], bf16)
    wl_d = din("wl", [P, HC], bf16)
    wr_d = din("wr", [P, HC], bf16)
    bl4_d = din("bl4", [1, 4 * HC], bf16)
    br4_d = din("br4", [1, 4 * HC], bf16)
    ones1_d = din("ones1", [1, P], bf16)
    idx_d = din("idx", [P, idx_cols], i16)
    eslot_d = din("eslot", [P, nt_tot], f32)
    iota_d = din("iota", [P, P], bf16)
    ident_d = din("ident", [P, P], bf16)
    att4_d = din("att4", [P, EGRP * HC], bf16)
    cb_d = din("cb", [P, P], f32)
    wq1_d = din("wq1_c", [HC, MLP_H], f32)
    wq2_d = din("wq2_c", [MLP_H, ACT_DIM], f32)
    wv1_d = din("wv1_c", [HC, MLP_H], f32)
    wv2_d = din("wv2_c", [MLP_H, 1], f32)
    wq2nm_d = din("wq2nm_c", [MLP_H, 1], f32)
    bq1_d = din("bq1_c", [MLP_H, 1], f32)
    bv1_d = din("bv1_c", [MLP_H, 1], f32)
    bq2_d = din("bq2_c", [ACT_DIM, 1], f32)
    ones110_d = din("ones110", [1, ACT_DIM], f32)
    cadd = float(inputs["bv2"][0] - inputs["bq2"].sum() / ACT_DIM)
    has_cb = bool(np.any(np.asarray(inputs["conv_bias"]) != 0))

    xl_d = nc.dram_tensor("xl_d", [NPAD, HC], bf16, kind="Internal").ap()
    out_q = nc.dram_tensor("out_q", [ACT_DIM, 8], f32, kind="ExternalOutput").ap()

    blk_chunks = {}
    for (b, j, lo_, hi_) in chunks:
        blk_chunks.setdefault(b, []).append((j, lo_, hi_))

    TMAX = max(t_lo[b] + t_hi[b] for b in range(NBLK))

    with tile.TileContext(nc) as tc, ExitStack() as ctx:
        cp = ctx.enter_context(tc.tile_pool(name="consts", bufs=1))

        def cload(name, ap_in, shape, dt):
            t = cp.tile(shape, dt, tag=name)
            nc.sync.dma_start(t[:], ap_in)
            return t

        iota_t = cload("iota", iota_d[:], [P, P], bf16)
        ident_t = cload("ident", ident_d[:], [P, P], bf16)
        att4_t = cload("att4", att4_d[:], [P, EGRP * HC], bf16)
        cb_t = cload("cb", cb_d[:], [P, P], f32) if has_cb else None
        wl_t = cload("wl", wl_d[:], [P, HC], bf16)
        wr_t = cload("wr", wr_d[:], [P, HC], bf16)
        bl4_t = cload("bl4", bl4_d[:], [1, 4 * HC], bf16)
        br4_t = cload("br4", br4_d[:], [1, 4 * HC], bf16)
        ones1_t = cload("ones1", ones1_d[:], [1, P], bf16)
        idx_t = cload("idx", idx_d[:], [P, idx_cols], i16)
        eslot_t = cload("eslot", eslot_d[:], [P, nt_tot], f32)
        wq1_t = cload("wq1", wq1_d[:], [HC, MLP_H], f32)
        wq2_t = cload("wq2", wq2_d[:], [MLP_H, ACT_DIM], f32)
        wv1_t = cload("wv1", wv1_d[:], [HC, MLP_H], f32)
        wv2_t = cload("wv2", wv2_d[:], [MLP_H, 1], f32)
        wq2nm_t = cload("wq2nm", wq2nm_d[:], [MLP_H, 1], f32)
        bq1_t = cload("bq1", bq1_d[:], [MLP_H, 1], f32)
        bv1_t = cload("bv1", bv1_d[:], [MLP_H, 1], f32)
        bq2_t = cload("bq2", bq2_d[:], [ACT_DIM, 1], f32)
        ones110_t = cload("ones110", ones110_d[:], [1, ACT_DIM], f32)

        # -------- phase 1b: xr for own nodes, resident in SBUF --------
        xr_res = [cp.tile([P, 4, HC], bf16, name=f"xr{i}", tag=f"xr{i}")
                  for i in range(XRB // 4)]
        p1l = ctx.enter_context(tc.tile_pool(name="p1l", bufs=3))
        p1o = ctx.enter_context(tc.tile_pool(name="p1o", bufs=3))
        with tc.tile_pool(name="p1p", bufs=2, space="PSUM") as p1p:
            for i in range(XRB // 4):
                lt = p1l.tile([P, 4 * P], bf16, tag="xrl")
                nc.sync.dma_start(lt[:], xrsrc_d[:, i * 4 * P:(i + 1) * 4 * P])
                pt = p1p.tile([P, 4 * HC], f32, tag="p1ps")
                nc.tensor.matmul(pt[:], ones1_t[:], br4_t[:], start=True, stop=False)
                for jj in range(4):
                    nc.tensor.matmul(pt[:, jj * HC:(jj + 1) * HC],
                                     lt[:, jj * P:(jj + 1) * P], wr_t[:],
                                     start=False, stop=(jj == 3))
                nc.scalar.activation(xr_res[i][:], pt[:], AF.Copy)

            # -------- phase 1: xl for all nodes -> DRAM bf16 --------
            for j in range(NPAD // (4 * P)):
                lt = p1l.tile([P, 4 * P], bf16, tag="xll")
                nc.sync.dma_start(lt[:], xT_d[:, j * 4 * P:(j + 1) * 4 * P])
                pt = p1p.tile([P, 4 * HC], f32, tag="p1ps")
                nc.tensor.matmul(pt[:], ones1_t[:], bl4_t[:], start=True, stop=False)
                for jj in range(4):
                    nc.tensor.matmul(pt[:, jj * HC:(jj + 1) * HC],
                                     lt[:, jj * P:(jj + 1) * P], wl_t[:],
                                     start=False, stop=(jj == 3))
                ot = p1o.tile([P, 4, HC], bf16, tag="p1out")
                nc.scalar.activation(ot[:], pt[:], AF.Copy)
                nc.sync.dma_start(
                    xl_d[j * 4 * P:(j + 1) * 4 * P, :]
                    .rearrange("(a b) c -> b a c", a=4),
                    ot[:])

        # ---------------- phase 2: edges ----------------
        gx = ctx.enter_context(tc.tile_pool(name="gx", bufs=3))
        wk = ctx.enter_context(tc.tile_pool(name="wk", bufs=3))
        fl = ctx.enter_context(tc.tile_pool(name="fl", bufs=2))
        pst_cm = tc.tile_pool(name="pst", bufs=2, space="PSUM")
        pst = pst_cm.__enter__()
        psx_cm = tc.tile_pool(name="psx", bufs=2, space="PSUM")
        psx = psx_cm.__enter__()
        agg_cm = tc.tile_pool(name="agg", bufs=2, space="PSUM")
        agg = agg_cm.__enter__()
        flp_cm = tc.tile_pool(name="flp", bufs=1, space="PSUM")
        flp = flp_cm.__enter__()

        gtmp = cp.tile([P, 8, 8], f32, tag="gtmp")
        nc.scalar.activation(gtmp[:].rearrange("p a b -> p (a b)"),
                             iota_t[:, 0:64], AF.Copy, bias=-3.0e38, scale=0.0)
        chunk_ctr = [0] * 8

        # gather tiles per block, prefetched via pool
        gxl = {}
        call_i = 0
        qrr = 0

        for b in range(NBLK):
            t_b = t_lo[b] + t_hi[b]
            gt = gx.tile([P, TMAX, HC], bf16, tag="gxl")
            # issue this block's gather calls
            while call_i < len(calls) and calls[call_i][2] == b:
                half, coff, _, toff, ct = calls[call_i]
                src_ap = xl_d[0:LO, :] if half == 0 else xl_d[LO:NPAD, :]
                nc.gpsimd.dma_gather(
                    gt[:, toff:toff + ct, :], src_ap,
                    idx_t[:, coff:coff + ct * P // 16],
                    ct * P, ct * P, HC, queue_num=qrr)
                qrr = (qrr + 1) % 4
                call_i += 1

            xr_blk = xr_res[b // 4][:, b % 4, :]
            ps_agg = agg.tile([P, HC + H], f32, tag="psagg")
            n_macro = (t_b + EGRP - 1) // EGRP
            for mm in range(n_macro):
                t0 = mm * EGRP
                nt_m = min(EGRP, t_b - t0)
                w = nt_m * P
                selm = wk.tile([P, EGRP * P], bf16, tag="selm")
                for g in range(nt_m):
                    nc.vector.tensor_scalar(
                        selm[:, g * P:(g + 1) * P], iota_t[:],
                        eslot_t[:, cum_t[b] + t0 + g: cum_t[b] + t0 + g + 1],
                        None, AL.is_equal)
                selT_ps = pst.tile([P, EGRP * P], bf16, tag="selT")
                for g in range(nt_m):
                    nc.tensor.transpose(selT_ps[:, g * P:(g + 1) * P],
                                        selm[:, g * P:(g + 1) * P], ident_t[:])
                selT = wk.tile([P, EGRP * P], bf16, tag="selTs")
                nc.scalar.activation(selT[:, :w], selT_ps[:, :w], AF.Copy)
                ps_x = psx.tile([P, EGRP * HC], f32, tag="psx")
                for g in range(nt_m):
                    nc.tensor.matmul(ps_x[:, g * HC:(g + 1) * HC],
                                     selT[:, g * P:(g + 1) * P], xr_blk,
                                     start=True, stop=False)
                    nc.tensor.matmul(ps_x[:, g * HC:(g + 1) * HC],
                                     ident_t[:], gt[:, t0 + g, :],
                                     start=False, stop=True)
                tm = wk.tile([P, EGRP * HC], bf16, tag="tm")
                nc.scalar.activation(tm[:, :nt_m * HC], ps_x[:, :nt_m * HC],
                                     AF.Prelu, alpha=NEG)
                um = wk.tile([P, EGRP * HC], bf16, tag="um")
                nc.vector.tensor_tensor(um[:, :nt_m * HC], tm[:, :nt_m * HC],
                                        att4_t[:, :nt_m * HC], op=AL.mult)
                em = wk.tile([P, EGRP * H], f32, tag="em")
                nc.vector.tensor_reduce(
                    em[:, :nt_m * H],
                    um[:, :nt_m * HC].rearrange("p (q c) -> p q c", c=C),
                    axis=mybir.AxisListType.X, op=AL.add)
                msgw = wk.tile([P, EGRP, HC + H], bf16, tag="msgw")
                nc.scalar.activation(
                    msgw[:, :nt_m, HC:HC + H],
                    em[:, :nt_m * H].rearrange("p (g h) -> p g h", h=H),
                    AF.Exp)
                nc.vector.tensor_tensor(
                    msgw[:, :nt_m, 0:HC].rearrange("p g (h c) -> p g h c", h=H),
                    gt[:, t0:t0 + nt_m, :].rearrange("p g (h c) -> p g h c", h=H),
                    msgw[:, :nt_m, HC:HC + H].to_broadcast([P, nt_m, H, C]),
                    op=AL.mult)
                for g in range(nt_m):
                    ti = t0 + g
                    nc.tensor.matmul(ps_agg[:], selm[:, g * P:(g + 1) * P],
                                     msgw[:, g, :], start=(ti == 0),
                                     stop=(ti == t_b - 1))
            # ---- flush block b ----
            rcp = fl.tile([P, H], f32, tag="rcp")
            nc.vector.reciprocal(rcp[:], ps_agg[:, HC:HC + H])
            outb = fl.tile([P, HC], bf16, tag="outb")
            if has_cb:
                outf = fl.tile([P, HC], f32, tag="outf")
                nc.vector.tensor_tensor(
                    outf[:].rearrange("p (h c) -> p h c", h=H),
                    ps_agg[:, 0:HC].rearrange("p (h c) -> p h c", h=H),
                    rcp[:].to_broadcast([P, H, C]), op=AL.mult)
                nc.vector.tensor_tensor(outb[:], outf[:], cb_t[:], op=AL.add)
            else:
                nc.vector.tensor_tensor(
                    outb[:].rearrange("p (h c) -> p h c", h=H),
                    ps_agg[:, 0:HC].rearrange("p (h c) -> p h c", h=H),
                    rcp[:].to_broadcast([P, H, C]), op=AL.mult)
            tp_ps = flp.tile([P, P], bf16, tag="tpps")
            nc.tensor.transpose(tp_ps[:], outb[:], ident_t[:])
            for (j, lo_, hi_) in blk_chunks.get(b, []):
                ci = chunk_ctr[j]
                chunk_ctr[j] += 1
                nc.vector.tensor_reduce(
                    gtmp[:, j, ci:ci + 1],
                    tp_ps[:, lo_:hi_], axis=mybir.AxisListType.X, op=AL.max)

        flp_cm.__exit__(None, None, None)
        agg_cm.__exit__(None, None, None)
        psx_cm.__exit__(None, None, None)
        pst_cm.__exit__(None, None, None)

        # ---------------- pooling + dueling head ----------------
        gacc = fl.tile([P, 8], f32, tag="gacc")
        nc.vector.tensor_reduce(gacc[:], gtmp[:], axis=mybir.AxisListType.X,
                                op=AL.max)
        grelu = fl.tile([P, 8], f32, tag="grelu")
        nc.scalar.activation(grelu[:], gacc[:], AF.Relu)

        with tc.tile_pool(name="mlp", bufs=1, space="PSUM") as mp:
            q1p = mp.tile([MLP_H, 8], f32, tag="q1p")
            nc.tensor.matmul(q1p[:], wq1_t[:], grelu[:], start=True, stop=True)
            q1s = fl.tile([MLP_H, 8], f32, tag="q1s")
            nc.scalar.activation(q1s[:], q1p[:], AF.Relu, bias=bq1_t[:, 0:1])
            v1p = mp.tile([MLP_H, 8], f32, tag="v1p")
            nc.tensor.matmul(v1p[:], wv1_t[:], grelu[:], start=True, stop=True)
            v1s = fl.tile([MLP_H, 8], f32, tag="v1s")
            nc.scalar.activation(v1s[:], v1p[:], AF.Relu, bias=bv1_t[:, 0:1])

            cvp = mp.tile([1, 8], f32, tag="cvp")
            nc.tensor.matmul(cvp[:], wv2_t[:], v1s[:], start=True, stop=False)
            nc.tensor.matmul(cvp[:], wq2nm_t[:], q1s[:], start=False, stop=True)
            corr = fl.tile([1, 8], f32, tag="corr")
            nc.vector.tensor_scalar(corr[:], cvp[:], cadd, None, AL.add)

            q2p = mp.tile([ACT_DIM, 8], f32, tag="q2p")
            nc.tensor.matmul(q2p[:], wq2_t[:], q1s[:], start=True, stop=False)
            nc.tensor.matmul(q2p[:], ones110_t[:], corr[:], start=False, stop=True)
            outsb = fl.tile([ACT_DIM, 8], f32, tag="outsb")
            nc.vector.tensor_scalar(outsb[:], q2p[:], bq2_t[:, 0:1], None, AL.add)
            nc.sync.dma_start(out_q[:], outsb[:])

    nc.compile()
    return nc


def kernel(**inputs):
    if _REPO not in sys.path:
        sys.path.insert(0, _REPO)
    from concourse.bass_utils import run_bass_kernel_spmd

    batch = inputs["batch"]
    assert np.array_equal(batch, ((np.arange(N) * G) // N).astype(batch.dtype))

    prep = _host_prep(inputs)
    (t_lo, t_hi, cum_t, nt_tot, calls, idx_cols,
     idx_all, eslot_all, xrsrc_all, xT, chunks) = prep
    nc = _build(inputs, prep)

    att_flat = np.asarray(inputs["att"], np.float32).reshape(-1)
    shared = dict(
        xT=_bf16(xT),
        wl=_bf16(inputs["Wl"]),
        wr=_bf16(inputs["Wr"]),
        bl4=_bf16(np.tile(inputs["bl"], 4))[None, :],
        br4=_bf16(np.tile(inputs["br"], 4))[None, :],
        ones1=_bf16(np.ones((1, P))),
        iota=_bf16(np.tile(np.arange(P, dtype=np.float32), (P, 1))),
        ident=_bf16(np.eye(P, dtype=np.float32)),
        att4=_bf16(np.tile(att_flat, (P, EGRP))),
        cb=np.tile(np.asarray(inputs["conv_bias"], np.float32), (P, 1)),
        wq1_c=np.asarray(inputs["Wq1"], np.float32),
        wq2_c=np.asarray(inputs["Wq2"], np.float32),
        wv1_c=np.asarray(inputs["Wv1"], np.float32),
        wv2_c=np.asarray(inputs["Wv2"], np.float32),
        wq2nm_c=(-np.asarray(inputs["Wq2"], np.float32).sum(1) / ACT_DIM)[:, None],
        bq1_c=np.asarray(inputs["bq1"], np.float32)[:, None],
        bv1_c=np.asarray(inputs["bv1"], np.float32)[:, None],
        bq2_c=np.asarray(inputs["bq2"], np.float32)[:, None],
        ones110=np.ones((1, ACT_DIM), np.float32),
    )
    in_maps = []
    for k in range(NCORES):
        m = dict(shared)
        m["idx"] = idx_all[k]
        m["eslot"] = np.asarray(eslot_all[k], np.float32)
        m["xrsrc"] = _bf16(xrsrc_all[k])
        in_maps.append(m)

    _cached["nc"] = nc
    _cached["in_maps"] = in_maps
    t0 = time.time()
    res = run_bass_kernel_spmd(nc, in_maps, core_ids=list(range(NCORES)))
    _timing["first_run_s"] = time.time() - t0
    t0 = time.time()
    res = run_bass_kernel_spmd(nc, in_maps, core_ids=list(range(NCORES)))
    _timing["second_run_s"] = time.time() - t0

    out = np.concatenate([res.results[k]["out_q"].T for k in range(NCORES)], axis=0)
    return out.astype(np.float32)


# revision 16
# speedup vs baseline: 205.2423x; 1.2240x over previous
"""GATv2 message-passing + dueling Q head on 8 Trainium2 NeuronCores.

Sharding: nodes (and incident edges, cut by destination) split into 8
contiguous ranges of 6250 nodes; graph boundaries align with core boundaries
so pooling and the MLP head run per-core. Each core computes xl = x@Wl for
ALL nodes in bf16 (written to DRAM) and xr = x@Wr for its OWN nodes (kept
resident in SBUF). Edges are grouped by destination block (128 dst slots);
per-edge xl rows are fetched with dma_gather (4 parallel SWDGE queues;
int16 indices force a lo/hi split of the node table), per-edge xr rows are
expanded on-chip via a PE matmul against the transposed slot-selection
matrix, and xl is accumulated into the same PSUM via an identity matmul.
Segment softmax runs without max-subtraction (scores are O(0.1)); weighted
messages and softmax denominators are scatter-added per block with one PE
matmul per edge tile.
"""
import sys
import math
import time
import numpy as np

_REPO = "/opt/trn_rl_repo"

N = 50000
E = 800000
G = 64
HC = 128
H = 4
C = 32
ACT_DIM = 10
MLP_H = 128
NEG = 0.2
NCORES = 8
NPC = N // NCORES            # 6250 nodes per core
P = 128
NBLK = math.ceil(NPC / P)    # 49 blocks of <=128 dst nodes
NPAD = 392 * P               # 50176 padded node count
LO = 32768                   # int16-addressable rows of xl_d
XRB = 52                     # own-node blocks padded to multiple of 4
EGRP = 4                     # edge tiles per macro
CALL_T = 8                   # tiles per dma_gather call (<=1024 idxs)

_timing = {}
_cached = {}


def rerun():
    """Re-run the last compiled kernel (for profiling from test.py)."""
    from concourse.bass_utils import run_bass_kernel_spmd
    return run_bass_kernel_spmd(_cached["nc"], _cached["in_maps"],
                                core_ids=list(range(NCORES)))


def _host_prep(inputs):
    ei = inputs["edge_index"].astype(np.int64)
    src = np.concatenate([ei[0], np.arange(N, dtype=np.int64)])
    dst = np.concatenate([ei[1], np.arange(N, dtype=np.int64)])
    core = dst // NPC

    # per (core, block): lo/hi edge lists (src, slot)
    per = [[None] * NBLK for _ in range(NCORES)]
    for k in range(NCORES):
        m = core == k
        s_k = src[m]
        d_k = dst[m] - k * NPC
        blk = d_k // P
        slot = d_k % P
        for b in range(NBLK):
            mb = blk == b
            sb = s_k[mb]
            sl = slot[mb]
            lo_m = sb < LO
            per[k][b] = ((sb[lo_m], sl[lo_m]), (sb[~lo_m] - LO, sl[~lo_m]))

    t_lo = [0] * NBLK
    t_hi = [0] * NBLK
    for b in range(NBLK):
        for k in range(NCORES):
            (slo, _), (shi, _) = per[k][b]
            t_lo[b] = max(t_lo[b], (len(slo) + P - 1) // P, 1)
            t_hi[b] = max(t_hi[b], (len(shi) + P - 1) // P, 1)

    # compile-time call list + tile layout (shared across cores)
    cum_t = [0] * (NBLK + 1)
    for b in range(NBLK):
        cum_t[b + 1] = cum_t[b] + t_lo[b] + t_hi[b]
    nt_tot = cum_t[NBLK]

    calls = []  # (half, idx_col_off, block, tile_off_in_block, ntiles)
    idx_cols = 0
    for b in range(NBLK):
        for half, tcnt, base in ((0, t_lo[b], 0), (1, t_hi[b], t_lo[b])):
            done = 0
            while done < tcnt:
                ct = min(CALL_T, tcnt - done)
                calls.append((half, idx_cols, b, base + done, ct))
                idx_cols += ct * P // 16
                done += ct

    # per-core tables
    idx_all, eslot_all, xrsrc_all = [], [], []
    xT = np.zeros((P, NPAD), np.float32)
    xT[:, :N] = np.asarray(inputs["x"], np.float32).T
    for k in range(NCORES):
        idx_flat = np.zeros(idx_cols * 16, np.int16)
        eslot = np.full((P, nt_tot), -1.0, np.float32)
        for b in range(NBLK):
            for half, tcnt, base in ((0, t_lo[b], 0), (1, t_hi[b], t_lo[b])):
                sb, sl = per[k][b][half]
                ne = len(sb)
                tile0 = cum_t[b] + base
                lanes = np.arange(ne)
                eslot[lanes % P, tile0 + lanes // P] = sl
                # write indices in tile order into the call regions
                # call entries for this (b, half) run start at the call list
                # offsets recorded above
                pos = 0
                for (h2, coff, b2, toff, ct) in calls:
                    if b2 != b or h2 != half or toff < base or toff >= base + tcnt:
                        continue
                    n_here = min(ne - pos, ct * P)
                    if n_here > 0:
                        idx_flat[coff * 16: coff * 16 + n_here] = sb[pos:pos + n_here]
                        pos += n_here
        # pack: entry i of each call region -> partition i%16, col i//16 (x8)
        packed = np.zeros((P, idx_cols), np.int16)
        for (h2, coff, b2, toff, ct) in calls:
            n = ct * P
            w = idx_flat[coff * 16: coff * 16 + n].reshape(n // 16, 16).T
            packed[:, coff: coff + n // 16] = np.tile(w, (8, 1))
        idx_all.append(packed)
        eslot_all.append(eslot)

        xs = np.zeros((P, XRB * P), np.float32)
        hi = min(N - k * NPC, XRB * P)
        xs[:, :hi] = xT[:, k * NPC: k * NPC + hi]
        xrsrc_all.append(xs)

    # graph chunk structure (identical on every core)
    lb = [int(math.ceil(N * j / G)) for j in range(9)]
    chunks = []   # (block, graph_j, lo, hi) node-local within block
    for b in range(NBLK):
        blo, bhi = b * P, min((b + 1) * P, NPC)
        for j in range(8):
            lo_, hi_ = max(lb[j], blo), min(lb[j + 1], bhi)
            if lo_ < hi_:
                chunks.append((b, j, lo_ - blo, hi_ - blo))
    return (t_lo, t_hi, cum_t, nt_tot, calls, idx_cols,
            idx_all, eslot_all, xrsrc_all, xT, chunks)


def _bf16(x):
    import ml_dtypes
    return np.asarray(x, np.float32).astype(ml_dtypes.bfloat16)


def _build(inputs, prep):
    if _REPO not in sys.path:
        sys.path.insert(0, _REPO)
    from contextlib import ExitStack
    import concourse.bass as bass
    import concourse.bacc as bacc
    import concourse.tile as tile
    from concourse import mybir

    (t_lo, t_hi, cum_t, nt_tot, calls, idx_cols,
     idx_all, eslot_all, xrsrc_all, xT, chunks) = prep

    f32 = mybir.dt.float32
    bf16 = mybir.dt.bfloat16
    i16 = mybir.dt.int16
    AL = mybir.AluOpType
    AF = mybir.ActivationFunctionType

    nc = bacc.Bacc("TRN2", target_bir_lowering=False, debug=False,
                   enable_asserts=False, num_devices=NCORES,
                   num_swdge_queues=4)

    def din(name, shape, dt):
        return nc.dram_tensor(name, shape, dt, kind="ExternalInput").ap()

    xT_d = din("xT", [P, NPAD], bf16)
    xrsrc_d = din("xrsrc", [P, XRB * P], bf16)
    wl_d = din("wl", [P, HC], bf16)
    wr_d = din("wr", [P, HC], bf16)
    bl4_d = din("bl4", [1, 4 * HC], bf16)
    br4_d = din("br4", [1, 4 * HC], bf16)
    ones1_d = din("ones1", [1, P], bf16)
    idx_d = din("idx", [P, idx_cols], i16)
    eslot_d = din("eslot", [P, nt_tot], bf16)
    iota_d = din("iota", [P, P], bf16)
    ident_d = din("ident", [P, P], bf16)
    att4_d = din("att4", [P, EGRP * HC], bf16)
    cb_d = din("cb", [P, P], f32)
    wq1_d = din("wq1_c", [HC, MLP_H], f32)
    wq2_d = din("wq2_c", [MLP_H, ACT_DIM], f32)
    wv1_d = din("wv1_c", [HC, MLP_H], f32)
    wv2_d = din("wv2_c", [MLP_H, 1], f32)
    wq2nm_d = din("wq2nm_c", [MLP_H, 1], f32)
    bq1_d = din("bq1_c", [MLP_H, 1], f32)
    bv1_d = din("bv1_c", [MLP_H, 1], f32)
    bq2_d = din("bq2_c", [ACT_DIM, 1], f32)
    ones110_d = din("ones110", [1, ACT_DIM], f32)
    cadd = float(inputs["bv2"][0] - inputs["bq2"].sum() / ACT_DIM)
    has_cb = bool(np.any(np.asarray(inputs["conv_bias"]) != 0))
    has_bl = bool(np.any(np.asarray(inputs["bl"]) != 0))
    has_br = bool(np.any(np.asarray(inputs["br"]) != 0))

    xl_d = nc.dram_tensor("xl_d", [NPAD, HC], bf16, kind="Internal").ap()
    out_q = nc.dram_tensor("out_q", [ACT_DIM, 8], f32, kind="ExternalOutput").ap()

    blk_chunks = {}
    for (b, j, lo_, hi_) in chunks:
        blk_chunks.setdefault(b, []).append((j, lo_, hi_))

    TMAX = max(t_lo[b] + t_hi[b] for b in range(NBLK))

    with tile.TileContext(nc) as tc, ExitStack() as ctx:
        cp = ctx.enter_context(tc.tile_pool(name="consts", bufs=1))

        def cload(name, ap_in, shape, dt):
            t = cp.tile(shape, dt, tag=name)
            nc.sync.dma_start(t[:], ap_in)
            return t

        iota_t = cload("iota", iota_d[:], [P, P], bf16)
        ident_t = cload("ident", ident_d[:], [P, P], bf16)
        att4_t = cload("att4", att4_d[:], [P, EGRP * HC], bf16)
        cb_t = cload("cb", cb_d[:], [P, P], f32) if has_cb else None
        wl_t = cload("wl", wl_d[:], [P, HC], bf16)
        wr_t = cload("wr", wr_d[:], [P, HC], bf16)
        bl4_t = cload("bl4", bl4_d[:], [1, 4 * HC], bf16)
        br4_t = cload("br4", br4_d[:], [1, 4 * HC], bf16)
        ones1_t = cload("ones1", ones1_d[:], [1, P], bf16)
        idx_t = cload("idx", idx_d[:], [P, idx_cols], i16)
        eslot_t = cload("eslot", eslot_d[:], [P, nt_tot], bf16)
        wq1_t = cload("wq1", wq1_d[:], [HC, MLP_H], f32)
        wq2_t = cload("wq2", wq2_d[:], [MLP_H, ACT_DIM], f32)
        wv1_t = cload("wv1", wv1_d[:], [HC, MLP_H], f32)
        wv2_t = cload("wv2", wv2_d[:], [MLP_H, 1], f32)
        wq2nm_t = cload("wq2nm", wq2nm_d[:], [MLP_H, 1], f32)
        bq1_t = cload("bq1", bq1_d[:], [MLP_H, 1], f32)
        bv1_t = cload("bv1", bv1_d[:], [MLP_H, 1], f32)
        bq2_t = cload("bq2", bq2_d[:], [ACT_DIM, 1], f32)
        ones110_t = cload("ones110", ones110_d[:], [1, ACT_DIM], f32)

        # -------- phase 1b: xr for own nodes, resident in SBUF --------
        xr_res = [cp.tile([P, 4, HC], bf16, name=f"xr{i}", tag=f"xr{i}")
                  for i in range(XRB // 4)]
        p1l = ctx.enter_context(tc.tile_pool(name="p1l", bufs=4))
        p1o = ctx.enter_context(tc.tile_pool(name="p1o", bufs=4))
        with tc.tile_pool(name="p1p", bufs=3, space="PSUM") as p1p:
            for i in range(XRB // 4):
                lt = p1l.tile([P, 4 * P], bf16, tag="xrl")
                nc.sync.dma_start(lt[:], xrsrc_d[:, i * 4 * P:(i + 1) * 4 * P])
                pt = p1p.tile([P, 4 * HC], f32, tag="p1ps")
                if has_br:
                    nc.tensor.matmul(pt[:], ones1_t[:], br4_t[:], start=True,
                                     stop=False)
                for jj in range(4):
                    nc.tensor.matmul(pt[:, jj * HC:(jj + 1) * HC],
                                     lt[:, jj * P:(jj + 1) * P], wr_t[:],
                                     start=not has_br, stop=(jj == 3))
                nc.scalar.activation(xr_res[i][:], pt[:], AF.Copy)

            # -------- phase 1: xl for all nodes -> DRAM bf16 --------
            for j in range(NPAD // (4 * P)):
                lt = p1l.tile([P, 4 * P], bf16, tag="xll")
                nc.sync.dma_start(lt[:], xT_d[:, j * 4 * P:(j + 1) * 4 * P])
                pt = p1p.tile([P, 4 * HC], f32, tag="p1ps")
                if has_bl:
                    nc.tensor.matmul(pt[:], ones1_t[:], bl4_t[:], start=True,
                                     stop=False)
                for jj in range(4):
                    nc.tensor.matmul(pt[:, jj * HC:(jj + 1) * HC],
                                     lt[:, jj * P:(jj + 1) * P], wl_t[:],
                                     start=not has_bl, stop=(jj == 3))
                ot = p1o.tile([P, 4, HC], bf16, tag="p1out")
                nc.scalar.activation(ot[:], pt[:], AF.Copy)
                nc.sync.dma_start(
                    xl_d[j * 4 * P:(j + 1) * 4 * P, :]
                    .rearrange("(a b) c -> b a c", a=4),
                    ot[:])

        # ---------------- phase 2: edges ----------------
        gx = ctx.enter_context(tc.tile_pool(name="gx", bufs=4))
        wk = ctx.enter_context(tc.tile_pool(name="wk", bufs=3))
        fl = ctx.enter_context(tc.tile_pool(name="fl", bufs=2))
        pst_cm = tc.tile_pool(name="pst", bufs=2, space="PSUM")
        pst = pst_cm.__enter__()
        psx_cm = tc.tile_pool(name="psx", bufs=2, space="PSUM")
        psx = psx_cm.__enter__()
        agg_cm = tc.tile_pool(name="agg", bufs=2, space="PSUM")
        agg = agg_cm.__enter__()
        flp_cm = tc.tile_pool(name="flp", bufs=1, space="PSUM")
        flp = flp_cm.__enter__()

        gtmp = cp.tile([P, 8, 8], f32, tag="gtmp")
        nc.scalar.activation(gtmp[:].rearrange("p a b -> p (a b)"),
                             iota_t[:, 0:64], AF.Copy, bias=-3.0e38, scale=0.0)
        chunk_ctr = [0] * 8

        # gather tiles per block, prefetched via pool
        gxl = {}
        call_i = 0
        qrr = 0

        for b in range(NBLK):
            t_b = t_lo[b] + t_hi[b]
            gt = gx.tile([P, TMAX, HC], bf16, tag="gxl")
            # issue this block's gather calls
            while call_i < len(calls) and calls[call_i][2] == b:
                half, coff, _, toff, ct = calls[call_i]
                src_ap = xl_d[0:LO, :] if half == 0 else xl_d[LO:NPAD, :]
                nc.gpsimd.dma_gather(
                    gt[:, toff:toff + ct, :], src_ap,
                    idx_t[:, coff:coff + ct * P // 16],
                    ct * P, ct * P, HC, queue_num=qrr)
                qrr = (qrr + 1) % 4
                call_i += 1

            xr_blk = xr_res[b // 4][:, b % 4, :]
            ps_agg = agg.tile([P, HC + H], f32, tag="psagg")
            n_macro = (t_b + EGRP - 1) // EGRP
            for mm in range(n_macro):
                t0 = mm * EGRP
                nt_m = min(EGRP, t_b - t0)
                w = nt_m * P
                selm = wk.tile([P, EGRP, P], bf16, tag="selm")
                c0 = cum_t[b] + t0
                nc.vector.tensor_tensor(
                    selm[:, :nt_m, :],
                    eslot_t[:, c0:c0 + nt_m].rearrange("p (g o) -> p g o", o=1)
                        .to_broadcast([P, nt_m, P]),
                    iota_t[:].rearrange("p (o f) -> p o f", o=1)
                        .to_broadcast([P, nt_m, P]),
                    op=AL.is_equal)
                selT_ps = pst.tile([P, EGRP * P], bf16, tag="selT")
                for g in range(nt_m):
                    nc.tensor.transpose(selT_ps[:, g * P:(g + 1) * P],
                                        selm[:, g, :], ident_t[:])
                selT = wk.tile([P, EGRP * P], bf16, tag="selTs")
                nc.scalar.activation(selT[:, :w], selT_ps[:, :w], AF.Copy)
                ps_x = psx.tile([P, EGRP * HC], f32, tag="psx")
                for g in range(nt_m):
                    nc.tensor.matmul(ps_x[:, g * HC:(g + 1) * HC],
                                     selT[:, g * P:(g + 1) * P], xr_blk,
                                     start=True, stop=False)
                    nc.tensor.matmul(ps_x[:, g * HC:(g + 1) * HC],
                                     ident_t[:], gt[:, t0 + g, :],
                                     start=False, stop=True)
                tm = wk.tile([P, EGRP * HC], bf16, tag="tm")
                nc.scalar.activation(tm[:, :nt_m * HC], ps_x[:, :nt_m * HC],
                                     AF.Prelu, alpha=NEG)
                um = wk.tile([P, EGRP * HC], bf16, tag="um")
                nc.vector.tensor_tensor(um[:, :nt_m * HC], tm[:, :nt_m * HC],
                                        att4_t[:, :nt_m * HC], op=AL.mult)
                em = wk.tile([P, EGRP * H], f32, tag="em")
                nc.vector.tensor_reduce(
                    em[:, :nt_m * H],
                    um[:, :nt_m * HC].rearrange("p (q c) -> p q c", c=C),
                    axis=mybir.AxisListType.X, op=AL.add)
                msgw = wk.tile([P, EGRP, HC + H], bf16, tag="msgw")
                nc.scalar.activation(
                    msgw[:, :nt_m, HC:HC + H],
                    em[:, :nt_m * H].rearrange("p (g h) -> p g h", h=H),
                    AF.Exp)
                nc.vector.tensor_tensor(
                    msgw[:, :nt_m, 0:HC].rearrange("p g (h c) -> p g h c", h=H),
                    gt[:, t0:t0 + nt_m, :].rearrange("p g (h c) -> p g h c", h=H),
                    msgw[:, :nt_m, HC:HC + H].to_broadcast([P, nt_m, H, C]),
                    op=AL.mult)
                for g in range(nt_m):
                    ti = t0 + g
                    nc.tensor.matmul(ps_agg[:], selm[:, g, :],
                                     msgw[:, g, :], start=(ti == 0),
                                     stop=(ti == t_b - 1))
            # ---- flush block b ----
            rcp = fl.tile([P, H], f32, tag="rcp")
            nc.vector.reciprocal(rcp[:], ps_agg[:, HC:HC + H])
            outb = fl.tile([P, HC], bf16, tag="outb")
            if has_cb:
                outf = fl.tile([P, HC], f32, tag="outf")
                nc.vector.tensor_tensor(
                    outf[:].rearrange("p (h c) -> p h c", h=H),
                    ps_agg[:, 0:HC].rearrange("p (h c) -> p h c", h=H),
                    rcp[:].to_broadcast([P, H, C]), op=AL.mult)
                nc.vector.tensor_tensor(outb[:], outf[:], cb_t[:], op=AL.add)
            else:
                nc.vector.tensor_tensor(
                    outb[:].rearrange("p (h c) -> p h c", h=H),
                    ps_agg[:, 0:HC].rearrange("p (h c) -> p h c", h=H),
                    rcp[:].to_broadcast([P, H, C]), op=AL.mult)
            tp_ps = flp.tile([P, P], bf16, tag="tpps")
            nc.tensor.transpose(tp_ps[:], outb[:], ident_t[:])
            for (j, lo_, hi_) in blk_chunks.get(b, []):
                ci = chunk_ctr[j]
                chunk_ctr[j] += 1
                nc.vector.tensor_reduce(
                    gtmp[:, j, ci:ci + 1],
                    tp_ps[:, lo_:hi_], axis=mybir.AxisListType.X, op=AL.max)

        flp_cm.__exit__(None, None, None)
        agg_cm.__exit__(None, None, None)
        psx_cm.__exit__(None, None, None)
        pst_cm.__exit__(None, None, None)

        # ---------------- pooling + dueling head ----------------
        gacc = fl.tile([P, 8], f32, tag="gacc")
        nc.vector.tensor_reduce(gacc[:], gtmp[:], axis=mybir.AxisListType.X,
                                op=AL.max)
        grelu = fl.tile([P, 8], f32, tag="grelu")
        nc.scalar.activation(grelu[:], gacc[:], AF.Relu)

        with tc.tile_pool(name="mlp", bufs=1, space="PSUM") as mp:
            q1p = mp.tile([MLP_H, 8], f32, tag="q1p")
            nc.tensor.matmul(q1p[:], wq1_t[:], grelu[:], start=True, stop=True)
            q1s = fl.tile([MLP_H, 8], f32, tag="q1s")
            nc.scalar.activation(q1s[:], q1p[:], AF.Relu, bias=bq1_t[:, 0:1])
            v1p = mp.tile([MLP_H, 8], f32, tag="v1p")
            nc.tensor.matmul(v1p[:], wv1_t[:], grelu[:], start=True, stop=True)
            v1s = fl.tile([MLP_H, 8], f32, tag="v1s")
            nc.scalar.activation(v1s[:], v1p[:], AF.Relu, bias=bv1_t[:, 0:1])

            cvp = mp.tile([1, 8], f32, tag="cvp")
            nc.tensor.matmul(cvp[:], wv2_t[:], v1s[:], start=True, stop=False)
            nc.tensor.matmul(cvp[:], wq2nm_t[:], q1s[:], start=False, stop=True)
            corr = fl.tile([1, 8], f32, tag="corr")
            nc.vector.tensor_scalar(corr[:], cvp[:], cadd, None, AL.add)

            q2p = mp.tile([ACT_DIM, 8], f32, tag="q2p")
            nc.tensor.matmul(q2p[:], wq2_t[:], q1s[:], start=True, stop=False)
            nc.tensor.matmul(q2p[:], ones110_t[:], corr[:], start=False, stop=True)
            outsb = fl.tile([ACT_DIM, 8], f32, tag="outsb")
            nc.vector.tensor_scalar(outsb[:], q2p[:], bq2_t[:, 0:1], None, AL.add)
            nc.sync.dma_start(out_q[:], outsb[:])

    nc.compile()
    return nc


def kernel(**inputs):
    if _REPO not in sys.path:
        sys.path.insert(0, _REPO)
    from concourse.bass_utils import run_bass_kernel_spmd

    batch = inputs["batch"]
    assert np.array_equal(batch, ((np.arange(N) * G) // N).astype(batch.dtype))

    prep = _host_prep(inputs)
    (t_lo, t_hi, cum_t, nt_tot, calls, idx_cols,
     idx_all, eslot_all, xrsrc_all, xT, chunks) = prep
    nc = _build(inputs, prep)

    att_flat = np.asarray(inputs["att"], np.float32).reshape(-1)
    shared = dict(
        xT=_bf16(xT),
        wl=_bf16(inputs["Wl"]),
        wr=_bf16(inputs["Wr"]),
        bl4=_bf16(np.tile(inputs["bl"], 4))[None, :],
        br4=_bf16(np.tile(inputs["br"], 4))[None, :],
        ones1=_bf16(np.ones((1, P))),
        iota=_bf16(np.tile(np.arange(P, dtype=np.float32), (P, 1))),
        ident=_bf16(np.eye(P, dtype=np.float32)),
        att4=_bf16(np.tile(att_flat, (P, EGRP))),
        cb=np.tile(np.asarray(inputs["conv_bias"], np.float32), (P, 1)),
        wq1_c=np.asarray(inputs["Wq1"], np.float32),
        wq2_c=np.asarray(inputs["Wq2"], np.float32),
        wv1_c=np.asarray(inputs["Wv1"], np.float32),
        wv2_c=np.asarray(inputs["Wv2"], np.float32),
        wq2nm_c=(-np.asarray(inputs["Wq2"], np.float32).sum(1) / ACT_DIM)[:, None],
        bq1_c=np.asarray(inputs["bq1"], np.float32)[:, None],
        bv1_c=np.asarray(inputs["bv1"], np.float32)[:, None],
        bq2_c=np.asarray(inputs["bq2"], np.float32)[:, None],
        ones110=np.ones((1, ACT_DIM), np.float32),
    )
    in_maps = []
    for k in range(NCORES):
        m = dict(shared)
        m["idx"] = idx_all[k]
        m["eslot"] = _bf16(eslot_all[k])
        m["xrsrc"] = _bf16(xrsrc_all[k])
        in_maps.append(m)

    _cached["nc"] = nc
    _cached["in_maps"] = in_maps
    t0 = time.time()
    res = run_bass_kernel_spmd(nc, in_maps, core_ids=list(range(NCORES)))
    _timing["first_run_s"] = time.time() - t0
    t0 = time.time()
    res = run_bass_kernel_spmd(nc, in_maps, core_ids=list(range(NCORES)))
    _timing["second_run_s"] = time.time() - t0

    out = np.concatenate([res.results[k]["out_q"].T for k in range(NCORES)], axis=0)
    return out.astype(np.float32)


# revision 17
# speedup vs baseline: 251.6208x; 1.2260x over previous
"""GATv2 message-passing + dueling Q head on 8 Trainium2 NeuronCores.

Sharding: nodes (and incident edges, cut by destination) split into 8
contiguous ranges of 6250 nodes; graph boundaries align with core boundaries
so pooling and the MLP head run per-core. Each core computes xl = x@Wl for
ALL nodes in bf16 (written to DRAM) and xr = x@Wr for its OWN nodes (kept
resident in SBUF). Edges are grouped by destination block (128 dst slots);
per-edge xl rows are fetched with dma_gather (4 parallel SWDGE queues;
int16 indices force a lo/hi split of the node table), per-edge xr rows are
expanded on-chip via a PE matmul against the transposed slot-selection
matrix, and xl is accumulated into the same PSUM via an identity matmul.
Segment softmax runs without max-subtraction (scores are O(0.1)); weighted
messages and softmax denominators are scatter-added per block with one PE
matmul per edge tile.
"""
import sys
import math
import time
import numpy as np

_REPO = "/opt/trn_rl_repo"

N = 50000
E = 800000
G = 64
HC = 128
H = 4
C = 32
ACT_DIM = 10
MLP_H = 128
NEG = 0.2
NCORES = 8
NPC = N // NCORES            # 6250 nodes per core
P = 128
NBLK = math.ceil(NPC / P)    # 49 blocks of <=128 dst nodes
NPAD = 392 * P               # 50176 padded node count
LO = 32768                   # int16-addressable rows of xl_d
XRB = 52                     # own-node blocks padded to multiple of 4
EGRP = 4                     # edge tiles per macro
CALL_T = 8                   # tiles per dma_gather call (<=1024 idxs)

_timing = {}
_cached = {}


def rerun():
    """Re-run the last compiled kernel (for profiling from test.py)."""
    from concourse.bass_utils import run_bass_kernel_spmd
    return run_bass_kernel_spmd(_cached["nc"], _cached["in_maps"],
                                core_ids=list(range(NCORES)))


def _host_prep(inputs):
    ei = inputs["edge_index"].astype(np.int64)
    src = np.concatenate([ei[0], np.arange(N, dtype=np.int64)])
    dst = np.concatenate([ei[1], np.arange(N, dtype=np.int64)])
    core = dst // NPC

    # per (core, block): lo/hi edge lists (src, slot)
    per = [[None] * NBLK for _ in range(NCORES)]
    for k in range(NCORES):
        m = core == k
        s_k = src[m]
        d_k = dst[m] - k * NPC
        blk = d_k // P
        slot = d_k % P
        for b in range(NBLK):
            mb = blk == b
            sb = s_k[mb]
            sl = slot[mb]
            lo_m = sb < LO
            per[k][b] = ((sb[lo_m], sl[lo_m]), (sb[~lo_m] - LO, sl[~lo_m]))

    t_lo = [0] * NBLK
    t_hi = [0] * NBLK
    for b in range(NBLK):
        for k in range(NCORES):
            (slo, _), (shi, _) = per[k][b]
            t_lo[b] = max(t_lo[b], (len(slo) + P - 1) // P, 1)
            t_hi[b] = max(t_hi[b], (len(shi) + P - 1) // P, 1)

    # compile-time call list + tile layout (shared across cores)
    cum_t = [0] * (NBLK + 1)
    for b in range(NBLK):
        cum_t[b + 1] = cum_t[b] + t_lo[b] + t_hi[b]
    nt_tot = cum_t[NBLK]

    calls = []  # (half, idx_col_off, block, tile_off_in_block, ntiles)
    idx_cols = 0
    for b in range(NBLK):
        for half, tcnt, base in ((0, t_lo[b], 0), (1, t_hi[b], t_lo[b])):
            done = 0
            while done < tcnt:
                ct = min(CALL_T, tcnt - done)
                calls.append((half, idx_cols, b, base + done, ct))
                idx_cols += ct * P // 16
                done += ct

    # per-core tables
    idx_all, eslot_all, xrsrc_all = [], [], []
    xT = np.zeros((P, NPAD), np.float32)
    xT[:, :N] = np.asarray(inputs["x"], np.float32).T
    for k in range(NCORES):
        idx_flat = np.zeros(idx_cols * 16, np.int16)
        eslot = np.full((P, nt_tot), -1.0, np.float32)
        for b in range(NBLK):
            for half, tcnt, base in ((0, t_lo[b], 0), (1, t_hi[b], t_lo[b])):
                sb, sl = per[k][b][half]
                ne = len(sb)
                tile0 = cum_t[b] + base
                lanes = np.arange(ne)
                eslot[lanes % P, tile0 + lanes // P] = sl
                # write indices in tile order into the call regions
                # call entries for this (b, half) run start at the call list
                # offsets recorded above
                pos = 0
                for (h2, coff, b2, toff, ct) in calls:
                    if b2 != b or h2 != half or toff < base or toff >= base + tcnt:
                        continue
                    n_here = min(ne - pos, ct * P)
                    if n_here > 0:
                        idx_flat[coff * 16: coff * 16 + n_here] = sb[pos:pos + n_here]
                        pos += n_here
        # pack: entry i of each call region -> partition i%16, col i//16 (x8)
        packed = np.zeros((P, idx_cols), np.int16)
        for (h2, coff, b2, toff, ct) in calls:
            n = ct * P
            w = idx_flat[coff * 16: coff * 16 + n].reshape(n // 16, 16).T
            packed[:, coff: coff + n // 16] = np.tile(w, (8, 1))
        idx_all.append(packed)
        eslot_all.append(eslot)

        xs = np.zeros((P, XRB * P), np.float32)
        hi = min(N - k * NPC, XRB * P)
        xs[:, :hi] = xT[:, k * NPC: k * NPC + hi]
        xrsrc_all.append(xs)

    # graph chunk structure (identical on every core)
    lb = [int(math.ceil(N * j / G)) for j in range(9)]
    chunks = []   # (block, graph_j, lo, hi) node-local within block
    for b in range(NBLK):
        blo, bhi = b * P, min((b + 1) * P, NPC)
        for j in range(8):
            lo_, hi_ = max(lb[j], blo), min(lb[j + 1], bhi)
            if lo_ < hi_:
                chunks.append((b, j, lo_ - blo, hi_ - blo))
    return (t_lo, t_hi, cum_t, nt_tot, calls, idx_cols,
            idx_all, eslot_all, xrsrc_all, xT, chunks)


def _bf16(x):
    import ml_dtypes
    return np.asarray(x, np.float32).astype(ml_dtypes.bfloat16)


def _build(inputs, prep):
    if _REPO not in sys.path:
        sys.path.insert(0, _REPO)
    from contextlib import ExitStack
    import concourse.bass as bass
    import concourse.bacc as bacc
    import concourse.tile as tile
    from concourse import mybir

    (t_lo, t_hi, cum_t, nt_tot, calls, idx_cols,
     idx_all, eslot_all, xrsrc_all, xT, chunks) = prep

    f32 = mybir.dt.float32
    bf16 = mybir.dt.bfloat16
    i16 = mybir.dt.int16
    AL = mybir.AluOpType
    AF = mybir.ActivationFunctionType

    nc = bacc.Bacc("TRN2", target_bir_lowering=False, debug=False,
                   enable_asserts=False, num_devices=NCORES,
                   num_swdge_queues=4)

    def din(name, shape, dt):
        return nc.dram_tensor(name, shape, dt, kind="ExternalInput").ap()

    xT_d = din("xT", [P, NPAD], bf16)
    xrsrc_d = din("xrsrc", [P, XRB * P], bf16)
    wl_d = din("wl", [P, HC], bf16)
    wr_d = din("wr", [P, HC], bf16)
    bl4_d = din("bl4", [1, 4 * HC], bf16)
    br4_d = din("br4", [1, 4 * HC], bf16)
    ones1_d = din("ones1", [1, P], bf16)
    idx_d = din("idx", [P, idx_cols], i16)
    eslot_d = din("eslot", [P, nt_tot], bf16)
    iota_d = din("iota", [P, P], bf16)
    ident_d = din("ident", [P, P], bf16)
    att4_d = din("att4", [P, EGRP * HC], bf16)
    cb_d = din("cb", [P, P], f32)
    wq1_d = din("wq1_c", [HC, MLP_H], f32)
    wq2_d = din("wq2_c", [MLP_H, ACT_DIM], f32)
    wv1_d = din("wv1_c", [HC, MLP_H], f32)
    wv2_d = din("wv2_c", [MLP_H, 1], f32)
    wq2nm_d = din("wq2nm_c", [MLP_H, 1], f32)
    bq1_d = din("bq1_c", [MLP_H, 1], f32)
    bv1_d = din("bv1_c", [MLP_H, 1], f32)
    bq2_d = din("bq2_c", [ACT_DIM, 1], f32)
    ones110_d = din("ones110", [1, ACT_DIM], f32)
    cadd = float(inputs["bv2"][0] - inputs["bq2"].sum() / ACT_DIM)
    has_cb = bool(np.any(np.asarray(inputs["conv_bias"]) != 0))
    has_bl = bool(np.any(np.asarray(inputs["bl"]) != 0))
    has_br = bool(np.any(np.asarray(inputs["br"]) != 0))

    xl_d = nc.dram_tensor("xl_d", [NPAD, HC], bf16, kind="Internal").ap()
    out_q = nc.dram_tensor("out_q", [ACT_DIM, 8], f32, kind="ExternalOutput").ap()

    blk_chunks = {}
    for (b, j, lo_, hi_) in chunks:
        blk_chunks.setdefault(b, []).append((j, lo_, hi_))

    TMAX = max(t_lo[b] + t_hi[b] for b in range(NBLK))

    with tile.TileContext(nc) as tc, ExitStack() as ctx:
        cp = ctx.enter_context(tc.tile_pool(name="consts", bufs=1))

        def cload(name, ap_in, shape, dt):
            t = cp.tile(shape, dt, tag=name)
            nc.sync.dma_start(t[:], ap_in)
            return t

        iota_t = cload("iota", iota_d[:], [P, P], bf16)
        ident_t = cload("ident", ident_d[:], [P, P], bf16)
        att4_t = cload("att4", att4_d[:], [P, EGRP * HC], bf16)
        cb_t = cload("cb", cb_d[:], [P, P], f32) if has_cb else None
        wl_t = cload("wl", wl_d[:], [P, HC], bf16)
        wr_t = cload("wr", wr_d[:], [P, HC], bf16)
        bl4_t = cload("bl4", bl4_d[:], [1, 4 * HC], bf16)
        br4_t = cload("br4", br4_d[:], [1, 4 * HC], bf16)
        ones1_t = cload("ones1", ones1_d[:], [1, P], bf16)
        idx_t = cload("idx", idx_d[:], [P, idx_cols], i16)
        eslot_t = cload("eslot", eslot_d[:], [P, nt_tot], bf16)
        wq1_t = cload("wq1", wq1_d[:], [HC, MLP_H], f32)
        wq2_t = cload("wq2", wq2_d[:], [MLP_H, ACT_DIM], f32)
        wv1_t = cload("wv1", wv1_d[:], [HC, MLP_H], f32)
        wv2_t = cload("wv2", wv2_d[:], [MLP_H, 1], f32)
        wq2nm_t = cload("wq2nm", wq2nm_d[:], [MLP_H, 1], f32)
        bq1_t = cload("bq1", bq1_d[:], [MLP_H, 1], f32)
        bv1_t = cload("bv1", bv1_d[:], [MLP_H, 1], f32)
        bq2_t = cload("bq2", bq2_d[:], [ACT_DIM, 1], f32)
        ones110_t = cload("ones110", ones110_d[:], [1, ACT_DIM], f32)

        # -------- phase 1b: xr for own nodes, resident in SBUF --------
        xr_res = [cp.tile([P, 4, HC], bf16, name=f"xr{i}", tag=f"xr{i}")
                  for i in range(XRB // 4)]
        p1l = ctx.enter_context(tc.tile_pool(name="p1l", bufs=4))
        p1o = ctx.enter_context(tc.tile_pool(name="p1o", bufs=4))
        with tc.tile_pool(name="p1p", bufs=3, space="PSUM") as p1p:
            for i in range(XRB // 4):
                lt = p1l.tile([P, 4 * P], bf16, tag="xrl")
                nc.sync.dma_start(lt[:], xrsrc_d[:, i * 4 * P:(i + 1) * 4 * P])
                pt = p1p.tile([P, 4 * HC], f32, tag="p1ps")
                if has_br:
                    nc.tensor.matmul(pt[:], ones1_t[:], br4_t[:], start=True,
                                     stop=False)
                for jj in range(4):
                    nc.tensor.matmul(pt[:, jj * HC:(jj + 1) * HC],
                                     lt[:, jj * P:(jj + 1) * P], wr_t[:],
                                     start=not has_br, stop=(jj == 3))
                nc.scalar.activation(xr_res[i][:], pt[:], AF.Copy)

            # -------- phase 1: xl for all nodes -> DRAM bf16 --------
            for j in range(NPAD // (8 * P)):
                lt = p1l.tile([P, 8 * P], bf16, tag="xll")
                nc.sync.dma_start(lt[:], xT_d[:, j * 8 * P:(j + 1) * 8 * P])
                ot = p1o.tile([P, 8, HC], bf16, tag="p1out")
                for hh in range(2):
                    pt = p1p.tile([P, 4 * HC], f32, tag="p1ps")
                    if has_bl:
                        nc.tensor.matmul(pt[:], ones1_t[:], bl4_t[:],
                                         start=True, stop=False)
                    for jj in range(4):
                        nc.tensor.matmul(pt[:, jj * HC:(jj + 1) * HC],
                                         lt[:, (hh * 4 + jj) * P:
                                            (hh * 4 + jj + 1) * P], wl_t[:],
                                         start=not has_bl, stop=(jj == 3))
                    if hh == 0:
                        nc.scalar.activation(
                            ot[:, 0:4].rearrange("p a c -> p (a c)"), pt[:],
                            AF.Copy)
                    else:
                        nc.vector.tensor_copy(
                            ot[:, 4:8].rearrange("p a c -> p (a c)"), pt[:])
                nc.scalar.dma_start(
                    xl_d[j * 8 * P:(j + 1) * 8 * P, :]
                    .rearrange("(a b) c -> b a c", a=8),
                    ot[:])

        # ---------------- phase 2: edges ----------------
        gx = ctx.enter_context(tc.tile_pool(name="gx", bufs=4))
        wk = ctx.enter_context(tc.tile_pool(name="wk", bufs=6))
        fl = ctx.enter_context(tc.tile_pool(name="fl", bufs=2))
        pst_cm = tc.tile_pool(name="pst", bufs=2, space="PSUM")
        pst = pst_cm.__enter__()
        psx_cm = tc.tile_pool(name="psx", bufs=2, space="PSUM")
        psx = psx_cm.__enter__()
        agg_cm = tc.tile_pool(name="agg", bufs=2, space="PSUM")
        agg = agg_cm.__enter__()
        flp_cm = tc.tile_pool(name="flp", bufs=1, space="PSUM")
        flp = flp_cm.__enter__()

        gtmp = cp.tile([P, 8, 8], f32, tag="gtmp")
        nc.scalar.activation(gtmp[:].rearrange("p a b -> p (a b)"),
                             iota_t[:, 0:64], AF.Copy, bias=-3.0e38, scale=0.0)
        chunk_ctr = [0] * 8

        # gather tiles per block, prefetched via pool
        gxl = {}
        call_i = 0
        qrr = 0

        for b in range(NBLK):
            t_b = t_lo[b] + t_hi[b]
            gt = gx.tile([P, TMAX, HC], bf16, tag="gxl")
            # issue this block's gather calls
            while call_i < len(calls) and calls[call_i][2] == b:
                half, coff, _, toff, ct = calls[call_i]
                src_ap = xl_d[0:LO, :] if half == 0 else xl_d[LO:NPAD, :]
                nc.gpsimd.dma_gather(
                    gt[:, toff:toff + ct, :], src_ap,
                    idx_t[:, coff:coff + ct * P // 16],
                    ct * P, ct * P, HC, queue_num=qrr)
                qrr = (qrr + 1) % 4
                call_i += 1

            xr_blk = xr_res[b // 4][:, b % 4, :]
            ps_agg = agg.tile([P, HC + H], f32, tag="psagg")
            n_macro = (t_b + EGRP - 1) // EGRP
            for mm in range(n_macro):
                t0 = mm * EGRP
                nt_m = min(EGRP, t_b - t0)
                w = nt_m * P
                selm = wk.tile([P, EGRP, P], bf16, tag="selm")
                c0 = cum_t[b] + t0
                nc.vector.tensor_tensor(
                    selm[:, :nt_m, :],
                    eslot_t[:, c0:c0 + nt_m].rearrange("p (g o) -> p g o", o=1)
                        .to_broadcast([P, nt_m, P]),
                    iota_t[:].rearrange("p (o f) -> p o f", o=1)
                        .to_broadcast([P, nt_m, P]),
                    op=AL.is_equal)
                selT_ps = pst.tile([P, EGRP * P], bf16, tag="selT")
                for g in range(nt_m):
                    nc.tensor.transpose(selT_ps[:, g * P:(g + 1) * P],
                                        selm[:, g, :], ident_t[:])
                selT = wk.tile([P, EGRP * P], bf16, tag="selTs")
                nc.scalar.activation(selT[:, :w], selT_ps[:, :w], AF.Copy)
                ps_x = psx.tile([P, EGRP * HC], f32, tag="psx")
                for g in range(nt_m):
                    nc.tensor.matmul(ps_x[:, g * HC:(g + 1) * HC],
                                     selT[:, g * P:(g + 1) * P], xr_blk,
                                     start=True, stop=False)
                    nc.tensor.matmul(ps_x[:, g * HC:(g + 1) * HC],
                                     ident_t[:], gt[:, t0 + g, :],
                                     start=False, stop=True)
                tm = wk.tile([P, EGRP * HC], bf16, tag="tm")
                nc.scalar.activation(tm[:, :nt_m * HC], ps_x[:, :nt_m * HC],
                                     AF.Prelu, alpha=NEG)
                um = wk.tile([P, EGRP * HC], bf16, tag="um")
                nc.vector.tensor_tensor(um[:, :nt_m * HC], tm[:, :nt_m * HC],
                                        att4_t[:, :nt_m * HC], op=AL.mult)
                em = wk.tile([P, EGRP * H], f32, tag="em")
                nc.vector.tensor_reduce(
                    em[:, :nt_m * H],
                    um[:, :nt_m * HC].rearrange("p (q c) -> p q c", c=C),
                    axis=mybir.AxisListType.X, op=AL.add)
                msgw = wk.tile([P, EGRP, HC + H], bf16, tag="msgw")
                nc.scalar.activation(
                    msgw[:, :nt_m, HC:HC + H],
                    em[:, :nt_m * H].rearrange("p (g h) -> p g h", h=H),
                    AF.Exp)
                wmx = wk.tile([P, EGRP, H, C], bf16, tag="wmx")
                nc.scalar.activation(
                    wmx[:, :nt_m],
                    em[:, :nt_m * H].rearrange("p (g h) -> p g h", h=H)
                        .rearrange("p g (h o) -> p g h o", o=1)
                        .to_broadcast([P, nt_m, H, C]),
                    AF.Exp)
                nc.vector.tensor_tensor(
                    msgw[:, :nt_m, 0:HC],
                    gt[:, t0:t0 + nt_m, :],
                    wmx[:, :nt_m].rearrange("p g h c -> p g (h c)"),
                    op=AL.mult)
                for g in range(nt_m):
                    ti = t0 + g
                    nc.tensor.matmul(ps_agg[:], selm[:, g, :],
                                     msgw[:, g, :], start=(ti == 0),
                                     stop=(ti == t_b - 1))
            # ---- flush block b ----
            rcp = fl.tile([P, H], f32, tag="rcp")
            nc.vector.reciprocal(rcp[:], ps_agg[:, HC:HC + H])
            outb = fl.tile([P, HC], bf16, tag="outb")
            if has_cb:
                outf = fl.tile([P, HC], f32, tag="outf")
                nc.vector.tensor_tensor(
                    outf[:].rearrange("p (h c) -> p h c", h=H),
                    ps_agg[:, 0:HC].rearrange("p (h c) -> p h c", h=H),
                    rcp[:].to_broadcast([P, H, C]), op=AL.mult)
                nc.vector.tensor_tensor(outb[:], outf[:], cb_t[:], op=AL.add)
            else:
                nc.vector.tensor_tensor(
                    outb[:].rearrange("p (h c) -> p h c", h=H),
                    ps_agg[:, 0:HC].rearrange("p (h c) -> p h c", h=H),
                    rcp[:].to_broadcast([P, H, C]), op=AL.mult)
            tp_ps = flp.tile([P, P], bf16, tag="tpps")
            nc.tensor.transpose(tp_ps[:], outb[:], ident_t[:])
            for (j, lo_, hi_) in blk_chunks.get(b, []):
                ci = chunk_ctr[j]
                chunk_ctr[j] += 1
                nc.vector.tensor_reduce(
                    gtmp[:, j, ci:ci + 1],
                    tp_ps[:, lo_:hi_], axis=mybir.AxisListType.X, op=AL.max)

        flp_cm.__exit__(None, None, None)
        agg_cm.__exit__(None, None, None)
        psx_cm.__exit__(None, None, None)
        pst_cm.__exit__(None, None, None)

        # ---------------- pooling + dueling head ----------------
        gacc = fl.tile([P, 8], f32, tag="gacc")
        nc.vector.tensor_reduce(gacc[:], gtmp[:], axis=mybir.AxisListType.X,
                                op=AL.max)
        grelu = fl.tile([P, 8], f32, tag="grelu")
        nc.scalar.activation(grelu[:], gacc[:], AF.Relu)

        with tc.tile_pool(name="mlp", bufs=1, space="PSUM") as mp:
            q1p = mp.tile([MLP_H, 8], f32, tag="q1p")
            nc.tensor.matmul(q1p[:], wq1_t[:], grelu[:], start=True, stop=True)
            q1s = fl.tile([MLP_H, 8], f32, tag="q1s")
            nc.scalar.activation(q1s[:], q1p[:], AF.Relu, bias=bq1_t[:, 0:1])
            v1p = mp.tile([MLP_H, 8], f32, tag="v1p")
            nc.tensor.matmul(v1p[:], wv1_t[:], grelu[:], start=True, stop=True)
            v1s = fl.tile([MLP_H, 8], f32, tag="v1s")
            nc.scalar.activation(v1s[:], v1p[:], AF.Relu, bias=bv1_t[:, 0:1])

            cvp = mp.tile([1, 8], f32, tag="cvp")
            nc.tensor.matmul(cvp[:], wv2_t[:], v1s[:], start=True, stop=False)
            nc.tensor.matmul(cvp[:], wq2nm_t[:], q1s[:], start=False, stop=True)
            corr = fl.tile([1, 8], f32, tag="corr")
            nc.vector.tensor_scalar(corr[:], cvp[:], cadd, None, AL.add)

            q2p = mp.tile([ACT_DIM, 8], f32, tag="q2p")
            nc.tensor.matmul(q2p[:], wq2_t[:], q1s[:], start=True, stop=False)
            nc.tensor.matmul(q2p[:], ones110_t[:], corr[:], start=False, stop=True)
            outsb = fl.tile([ACT_DIM, 8], f32, tag="outsb")
            nc.vector.tensor_scalar(outsb[:], q2p[:], bq2_t[:, 0:1], None, AL.add)
            nc.sync.dma_start(out_q[:], outsb[:])

    nc.compile()
    return nc


def kernel(**inputs):
    if _REPO not in sys.path:
        sys.path.insert(0, _REPO)
    from concourse.bass_utils import run_bass_kernel_spmd

    batch = inputs["batch"]
    assert np.array_equal(batch, ((np.arange(N) * G) // N).astype(batch.dtype))

    prep = _host_prep(inputs)
    (t_lo, t_hi, cum_t, nt_tot, calls, idx_cols,
     idx_all, eslot_all, xrsrc_all, xT, chunks) = prep
    nc = _build(inputs, prep)

    att_flat = np.asarray(inputs["att"], np.float32).reshape(-1)
    shared = dict(
        xT=_bf16(xT),
        wl=_bf16(inputs["Wl"]),
        wr=_bf16(inputs["Wr"]),
        bl4=_bf16(np.tile(inputs["bl"], 4))[None, :],
        br4=_bf16(np.tile(inputs["br"], 4))[None, :],
        ones1=_bf16(np.ones((1, P))),
        iota=_bf16(np.tile(np.arange(P, dtype=np.float32), (P, 1))),
        ident=_bf16(np.eye(P, dtype=np.float32)),
        att4=_bf16(np.tile(att_flat, (P, EGRP))),
        cb=np.tile(np.asarray(inputs["conv_bias"], np.float32), (P, 1)),
        wq1_c=np.asarray(inputs["Wq1"], np.float32),
        wq2_c=np.asarray(inputs["Wq2"], np.float32),
        wv1_c=np.asarray(inputs["Wv1"], np.float32),
        wv2_c=np.asarray(inputs["Wv2"], np.float32),
        wq2nm_c=(-np.asarray(inputs["Wq2"], np.float32).sum(1) / ACT_DIM)[:, None],
        bq1_c=np.asarray(inputs["bq1"], np.float32)[:, None],
        bv1_c=np.asarray(inputs["bv1"], np.float32)[:, None],
        bq2_c=np.asarray(inputs["bq2"], np.float32)[:, None],
        ones110=np.ones((1, ACT_DIM), np.float32),
    )
    in_maps = []
    for k in range(NCORES):
        m = dict(shared)
        m["idx"] = idx_all[k]
        m["eslot"] = _bf16(eslot_all[k])
        m["xrsrc"] = _bf16(xrsrc_all[k])
        in_maps.append(m)

    _cached["nc"] = nc
    _cached["in_maps"] = in_maps
    t0 = time.time()
    res = run_bass_kernel_spmd(nc, in_maps, core_ids=list(range(NCORES)))
    _timing["first_run_s"] = time.time() - t0
    t0 = time.time()
    res = run_bass_kernel_spmd(nc, in_maps, core_ids=list(range(NCORES)))
    _timing["second_run_s"] = time.time() - t0

    out = np.concatenate([res.results[k]["out_q"].T for k in range(NCORES)], axis=0)
    return out.astype(np.float32)


# revision 18
# speedup vs baseline: 253.4366x; 1.0072x over previous
"""GATv2 message-passing + dueling Q head on 8 Trainium2 NeuronCores.

Sharding: nodes (and incident edges, cut by destination) split into 8
contiguous ranges of 6250 nodes; graph boundaries align with core boundaries
so pooling and the MLP head run per-core. Each core computes xl = x@Wl for
ALL nodes in bf16 (written to DRAM) and xr = x@Wr for its OWN nodes (kept
resident in SBUF). Edges are grouped by destination block (128 dst slots);
per-edge xl rows are fetched with dma_gather (4 parallel SWDGE queues;
int16 indices force a lo/hi split of the node table), per-edge xr rows are
expanded on-chip via a PE matmul against the transposed slot-selection
matrix, and xl is accumulated into the same PSUM via an identity matmul.
Segment softmax runs without max-subtraction (scores are O(0.1)); weighted
messages and softmax denominators are scatter-added per block with one PE
matmul per edge tile.
"""
import sys
import math
import time
import numpy as np

_REPO = "/opt/trn_rl_repo"

N = 50000
E = 800000
G = 64
HC = 128
H = 4
C = 32
ACT_DIM = 10
MLP_H = 128
NEG = 0.2
NCORES = 8
NPC = N // NCORES            # 6250 nodes per core
P = 128
NBLK = math.ceil(NPC / P)    # 49 blocks of <=128 dst nodes
NPAD = 392 * P               # 50176 padded node count
LO = 32768                   # int16-addressable rows of xl_d
XRB = 52                     # own-node blocks padded to multiple of 4
EGRP = 6                     # edge tiles per macro
CALL_T = 8                   # tiles per dma_gather call (<=1024 idxs)

_timing = {}
_cached = {}


def rerun():
    """Re-run the last compiled kernel (for profiling from test.py)."""
    from concourse.bass_utils import run_bass_kernel_spmd
    return run_bass_kernel_spmd(_cached["nc"], _cached["in_maps"],
                                core_ids=list(range(NCORES)))


def _host_prep(inputs):
    ei = inputs["edge_index"].astype(np.int64)
    src = np.concatenate([ei[0], np.arange(N, dtype=np.int64)])
    dst = np.concatenate([ei[1], np.arange(N, dtype=np.int64)])
    core = dst // NPC

    # per (core, block): lo/hi edge lists (src, slot)
    per = [[None] * NBLK for _ in range(NCORES)]
    for k in range(NCORES):
        m = core == k
        s_k = src[m]
        d_k = dst[m] - k * NPC
        blk = d_k // P
        slot = d_k % P
        for b in range(NBLK):
            mb = blk == b
            sb = s_k[mb]
            sl = slot[mb]
            lo_m = sb < LO
            per[k][b] = ((sb[lo_m], sl[lo_m]), (sb[~lo_m] - LO, sl[~lo_m]))

    t_lo = [0] * NBLK
    t_hi = [0] * NBLK
    for b in range(NBLK):
        for k in range(NCORES):
            (slo, _), (shi, _) = per[k][b]
            t_lo[b] = max(t_lo[b], (len(slo) + P - 1) // P, 1)
            t_hi[b] = max(t_hi[b], (len(shi) + P - 1) // P, 1)

    # compile-time call list + tile layout (shared across cores)
    cum_t = [0] * (NBLK + 1)
    for b in range(NBLK):
        cum_t[b + 1] = cum_t[b] + t_lo[b] + t_hi[b]
    nt_tot = cum_t[NBLK]

    calls = []  # (half, idx_col_off, block, tile_off_in_block, ntiles)
    idx_cols = 0
    for b in range(NBLK):
        for half, tcnt, base in ((0, t_lo[b], 0), (1, t_hi[b], t_lo[b])):
            done = 0
            while done < tcnt:
                ct = min(CALL_T, tcnt - done)
                calls.append((half, idx_cols, b, base + done, ct))
                idx_cols += ct * P // 16
                done += ct

    # per-core tables
    idx_all, eslot_all, xrsrc_all = [], [], []
    xT = np.zeros((P, NPAD), np.float32)
    xT[:, :N] = np.asarray(inputs["x"], np.float32).T
    for k in range(NCORES):
        idx_flat = np.zeros(idx_cols * 16, np.int16)
        eslot = np.full((P, nt_tot), -1.0, np.float32)
        for b in range(NBLK):
            for half, tcnt, base in ((0, t_lo[b], 0), (1, t_hi[b], t_lo[b])):
                sb, sl = per[k][b][half]
                ne = len(sb)
                tile0 = cum_t[b] + base
                lanes = np.arange(ne)
                eslot[lanes % P, tile0 + lanes // P] = sl
                # write indices in tile order into the call regions
                # call entries for this (b, half) run start at the call list
                # offsets recorded above
                pos = 0
                for (h2, coff, b2, toff, ct) in calls:
                    if b2 != b or h2 != half or toff < base or toff >= base + tcnt:
                        continue
                    n_here = min(ne - pos, ct * P)
                    if n_here > 0:
                        idx_flat[coff * 16: coff * 16 + n_here] = sb[pos:pos + n_here]
                        pos += n_here
        # pack: entry i of each call region -> partition i%16, col i//16 (x8)
        packed = np.zeros((P, idx_cols), np.int16)
        for (h2, coff, b2, toff, ct) in calls:
            n = ct * P
            w = idx_flat[coff * 16: coff * 16 + n].reshape(n // 16, 16).T
            packed[:, coff: coff + n // 16] = np.tile(w, (8, 1))
        idx_all.append(packed)
        eslot_all.append(eslot)

        xs = np.zeros((P, XRB * P), np.float32)
        hi = min(N - k * NPC, XRB * P)
        xs[:, :hi] = xT[:, k * NPC: k * NPC + hi]
        xrsrc_all.append(xs)

    # graph chunk structure (identical on every core)
    lb = [int(math.ceil(N * j / G)) for j in range(9)]
    chunks = []   # (block, graph_j, lo, hi) node-local within block
    for b in range(NBLK):
        blo, bhi = b * P, min((b + 1) * P, NPC)
        for j in range(8):
            lo_, hi_ = max(lb[j], blo), min(lb[j + 1], bhi)
            if lo_ < hi_:
                chunks.append((b, j, lo_ - blo, hi_ - blo))
    return (t_lo, t_hi, cum_t, nt_tot, calls, idx_cols,
            idx_all, eslot_all, xrsrc_all, xT, chunks)


def _bf16(x):
    import ml_dtypes
    return np.asarray(x, np.float32).astype(ml_dtypes.bfloat16)


def _build(inputs, prep):
    if _REPO not in sys.path:
        sys.path.insert(0, _REPO)
    from contextlib import ExitStack
    import concourse.bass as bass
    import concourse.bacc as bacc
    import concourse.tile as tile
    from concourse import mybir

    (t_lo, t_hi, cum_t, nt_tot, calls, idx_cols,
     idx_all, eslot_all, xrsrc_all, xT, chunks) = prep

    f32 = mybir.dt.float32
    bf16 = mybir.dt.bfloat16
    i16 = mybir.dt.int16
    AL = mybir.AluOpType
    AF = mybir.ActivationFunctionType

    nc = bacc.Bacc("TRN2", target_bir_lowering=False, debug=False,
                   enable_asserts=False, num_devices=NCORES,
                   num_swdge_queues=4)

    def din(name, shape, dt):
        return nc.dram_tensor(name, shape, dt, kind="ExternalInput").ap()

    xT_d = din("xT", [P, NPAD], bf16)
    xrsrc_d = din("xrsrc", [P, XRB * P], bf16)
    wl_d = din("wl", [P, HC], bf16)
    wr_d = din("wr", [P, HC], bf16)
    bl4_d = din("bl4", [1, 4 * HC], bf16)
    br4_d = din("br4", [1, 4 * HC], bf16)
    ones1_d = din("ones1", [1, P], bf16)
    idx_d = din("idx", [P, idx_cols], i16)
    eslot_d = din("eslot", [P, nt_tot], bf16)
    iota_d = din("iota", [P, P], bf16)
    ident_d = din("ident", [P, P], bf16)
    att4_d = din("att4", [P, EGRP * HC], bf16)
    cb_d = din("cb", [P, P], f32)
    wq1_d = din("wq1_c", [HC, MLP_H], f32)
    wq2_d = din("wq2_c", [MLP_H, ACT_DIM], f32)
    wv1_d = din("wv1_c", [HC, MLP_H], f32)
    wv2_d = din("wv2_c", [MLP_H, 1], f32)
    wq2nm_d = din("wq2nm_c", [MLP_H, 1], f32)
    bq1_d = din("bq1_c", [MLP_H, 1], f32)
    bv1_d = din("bv1_c", [MLP_H, 1], f32)
    bq2_d = din("bq2_c", [ACT_DIM, 1], f32)
    ones110_d = din("ones110", [1, ACT_DIM], f32)
    cadd = float(inputs["bv2"][0] - inputs["bq2"].sum() / ACT_DIM)
    has_cb = bool(np.any(np.asarray(inputs["conv_bias"]) != 0))
    has_bl = bool(np.any(np.asarray(inputs["bl"]) != 0))
    has_br = bool(np.any(np.asarray(inputs["br"]) != 0))

    xl_d = nc.dram_tensor("xl_d", [NPAD, HC], bf16, kind="Internal").ap()
    out_q = nc.dram_tensor("out_q", [ACT_DIM, 8], f32, kind="ExternalOutput").ap()

    blk_chunks = {}
    for (b, j, lo_, hi_) in chunks:
        blk_chunks.setdefault(b, []).append((j, lo_, hi_))

    TMAX = max(t_lo[b] + t_hi[b] for b in range(NBLK))

    with tile.TileContext(nc) as tc, ExitStack() as ctx:
        cp = ctx.enter_context(tc.tile_pool(name="consts", bufs=1))

        def cload(name, ap_in, shape, dt):
            t = cp.tile(shape, dt, tag=name)
            nc.sync.dma_start(t[:], ap_in)
            return t

        iota_t = cload("iota", iota_d[:], [P, P], bf16)
        ident_t = cload("ident", ident_d[:], [P, P], bf16)
        att4_t = cload("att4", att4_d[:], [P, EGRP * HC], bf16)
        cb_t = cload("cb", cb_d[:], [P, P], f32) if has_cb else None
        wl_t = cload("wl", wl_d[:], [P, HC], bf16)
        wr_t = cload("wr", wr_d[:], [P, HC], bf16)
        bl4_t = cload("bl4", bl4_d[:], [1, 4 * HC], bf16)
        br4_t = cload("br4", br4_d[:], [1, 4 * HC], bf16)
        ones1_t = cload("ones1", ones1_d[:], [1, P], bf16)
        idx_t = cload("idx", idx_d[:], [P, idx_cols], i16)
        eslot_t = cload("eslot", eslot_d[:], [P, nt_tot], bf16)
        wq1_t = cload("wq1", wq1_d[:], [HC, MLP_H], f32)
        wq2_t = cload("wq2", wq2_d[:], [MLP_H, ACT_DIM], f32)
        wv1_t = cload("wv1", wv1_d[:], [HC, MLP_H], f32)
        wv2_t = cload("wv2", wv2_d[:], [MLP_H, 1], f32)
        wq2nm_t = cload("wq2nm", wq2nm_d[:], [MLP_H, 1], f32)
        bq1_t = cload("bq1", bq1_d[:], [MLP_H, 1], f32)
        bv1_t = cload("bv1", bv1_d[:], [MLP_H, 1], f32)
        bq2_t = cload("bq2", bq2_d[:], [ACT_DIM, 1], f32)
        ones110_t = cload("ones110", ones110_d[:], [1, ACT_DIM], f32)

        # -------- phase 1b: xr for own nodes, resident in SBUF --------
        xr_res = [cp.tile([P, 4, HC], bf16, name=f"xr{i}", tag=f"xr{i}")
                  for i in range(XRB // 4)]
        p1l = ctx.enter_context(tc.tile_pool(name="p1l", bufs=4))
        p1o = ctx.enter_context(tc.tile_pool(name="p1o", bufs=4))
        with tc.tile_pool(name="p1p", bufs=3, space="PSUM") as p1p:
            for i in range(XRB // 4):
                lt = p1l.tile([P, 4 * P], bf16, tag="xrl")
                nc.sync.dma_start(lt[:], xrsrc_d[:, i * 4 * P:(i + 1) * 4 * P])
                pt = p1p.tile([P, 4 * HC], f32, tag="p1ps")
                if has_br:
                    nc.tensor.matmul(pt[:], ones1_t[:], br4_t[:], start=True,
                                     stop=False)
                for jj in range(4):
                    nc.tensor.matmul(pt[:, jj * HC:(jj + 1) * HC],
                                     lt[:, jj * P:(jj + 1) * P], wr_t[:],
                                     start=not has_br, stop=(jj == 3))
                nc.scalar.activation(xr_res[i][:], pt[:], AF.Copy)

            # -------- phase 1: xl for all nodes -> DRAM bf16 --------
            for j in range(NPAD // (8 * P)):
                lt = p1l.tile([P, 8 * P], bf16, tag="xll")
                nc.sync.dma_start(lt[:], xT_d[:, j * 8 * P:(j + 1) * 8 * P])
                ot = p1o.tile([P, 8, HC], bf16, tag="p1out")
                for hh in range(2):
                    pt = p1p.tile([P, 4 * HC], f32, tag="p1ps")
                    if has_bl:
                        nc.tensor.matmul(pt[:], ones1_t[:], bl4_t[:],
                                         start=True, stop=False)
                    for jj in range(4):
                        nc.tensor.matmul(pt[:, jj * HC:(jj + 1) * HC],
                                         lt[:, (hh * 4 + jj) * P:
                                            (hh * 4 + jj + 1) * P], wl_t[:],
                                         start=not has_bl, stop=(jj == 3))
                    nc.vector.tensor_copy(
                        ot[:, hh * 4:(hh + 1) * 4]
                        .rearrange("p a c -> p (a c)"), pt[:])
                nc.scalar.dma_start(
                    xl_d[j * 8 * P:(j + 1) * 8 * P, :]
                    .rearrange("(a b) c -> b a c", a=8),
                    ot[:])

        # ---------------- phase 2: edges ----------------
        gx = ctx.enter_context(tc.tile_pool(name="gx", bufs=4))
        wk = ctx.enter_context(tc.tile_pool(name="wk", bufs=6))
        fl = ctx.enter_context(tc.tile_pool(name="fl", bufs=2))
        pst_cm = tc.tile_pool(name="pst", bufs=1, space="PSUM")
        pst = pst_cm.__enter__()
        psx_cm = tc.tile_pool(name="psx", bufs=2, space="PSUM")
        psx = psx_cm.__enter__()
        agg_cm = tc.tile_pool(name="agg", bufs=2, space="PSUM")
        agg = agg_cm.__enter__()
        flp_cm = tc.tile_pool(name="flp", bufs=1, space="PSUM")
        flp = flp_cm.__enter__()

        gtmp = cp.tile([P, 8, 8], f32, tag="gtmp")
        nc.scalar.activation(gtmp[:].rearrange("p a b -> p (a b)"),
                             iota_t[:, 0:64], AF.Copy, bias=-3.0e38, scale=0.0)
        chunk_ctr = [0] * 8

        # gather tiles per block, prefetched via pool
        gxl = {}
        call_i = 0
        qrr = 0

        for b in range(NBLK):
            t_b = t_lo[b] + t_hi[b]
            gt = gx.tile([P, TMAX, HC], bf16, tag="gxl")
            # issue this block's gather calls
            while call_i < len(calls) and calls[call_i][2] == b:
                half, coff, _, toff, ct = calls[call_i]
                src_ap = xl_d[0:LO, :] if half == 0 else xl_d[LO:NPAD, :]
                nc.gpsimd.dma_gather(
                    gt[:, toff:toff + ct, :], src_ap,
                    idx_t[:, coff:coff + ct * P // 16],
                    ct * P, ct * P, HC, queue_num=qrr)
                qrr = (qrr + 1) % 4
                call_i += 1

            xr_blk = xr_res[b // 4][:, b % 4, :]
            ps_agg = agg.tile([P, HC + H], f32, tag="psagg")
            n_macro = (t_b + EGRP - 1) // EGRP
            for mm in range(n_macro):
                t0 = mm * EGRP
                nt_m = min(EGRP, t_b - t0)
                w = nt_m * P
                selm = wk.tile([P, EGRP, P], bf16, tag="selm")
                c0 = cum_t[b] + t0
                nc.vector.tensor_tensor(
                    selm[:, :nt_m, :],
                    eslot_t[:, c0:c0 + nt_m].rearrange("p (g o) -> p g o", o=1)
                        .to_broadcast([P, nt_m, P]),
                    iota_t[:].rearrange("p (o f) -> p o f", o=1)
                        .to_broadcast([P, nt_m, P]),
                    op=AL.is_equal)
                selT_ps = pst.tile([P, EGRP * P], bf16, tag="selT")
                for g in range(nt_m):
                    nc.tensor.transpose(selT_ps[:, g * P:(g + 1) * P],
                                        selm[:, g, :], ident_t[:])
                selT = wk.tile([P, EGRP * P], bf16, tag="selTs")
                nc.scalar.activation(selT[:, :w], selT_ps[:, :w], AF.Copy)
                ps_x = psx.tile([P, EGRP * HC], f32, tag="psx")
                for g in range(nt_m):
                    nc.tensor.matmul(ps_x[:, g * HC:(g + 1) * HC],
                                     selT[:, g * P:(g + 1) * P], xr_blk,
                                     start=True, stop=False)
                    nc.tensor.matmul(ps_x[:, g * HC:(g + 1) * HC],
                                     ident_t[:], gt[:, t0 + g, :],
                                     start=False, stop=True)
                tm = wk.tile([P, EGRP * HC], bf16, tag="tm")
                nc.scalar.activation(tm[:, :nt_m * HC], ps_x[:, :nt_m * HC],
                                     AF.Prelu, alpha=NEG)
                um = wk.tile([P, EGRP * HC], bf16, tag="um")
                nc.vector.tensor_tensor(um[:, :nt_m * HC], tm[:, :nt_m * HC],
                                        att4_t[:, :nt_m * HC], op=AL.mult)
                em = wk.tile([P, EGRP * H], f32, tag="em")
                nc.vector.tensor_reduce(
                    em[:, :nt_m * H],
                    um[:, :nt_m * HC].rearrange("p (q c) -> p q c", c=C),
                    axis=mybir.AxisListType.X, op=AL.add)
                msgw = wk.tile([P, EGRP, HC + H], bf16, tag="msgw")
                nc.scalar.activation(
                    msgw[:, :nt_m, HC:HC + H],
                    em[:, :nt_m * H].rearrange("p (g h) -> p g h", h=H),
                    AF.Exp)
                wmx = wk.tile([P, EGRP, H, C], bf16, tag="wmx")
                nc.scalar.activation(
                    wmx[:, :nt_m],
                    em[:, :nt_m * H].rearrange("p (g h) -> p g h", h=H)
                        .rearrange("p g (h o) -> p g h o", o=1)
                        .to_broadcast([P, nt_m, H, C]),
                    AF.Exp)
                nc.vector.tensor_tensor(
                    msgw[:, :nt_m, 0:HC],
                    gt[:, t0:t0 + nt_m, :],
                    wmx[:, :nt_m].rearrange("p g h c -> p g (h c)"),
                    op=AL.mult)
                for g in range(nt_m):
                    ti = t0 + g
                    nc.tensor.matmul(ps_agg[:], selm[:, g, :],
                                     msgw[:, g, :], start=(ti == 0),
                                     stop=(ti == t_b - 1))
            # ---- flush block b ----
            rcp = fl.tile([P, H], f32, tag="rcp")
            nc.vector.reciprocal(rcp[:], ps_agg[:, HC:HC + H])
            outb = fl.tile([P, HC], bf16, tag="outb")
            if has_cb:
                outf = fl.tile([P, HC], f32, tag="outf")
                nc.vector.tensor_tensor(
                    outf[:].rearrange("p (h c) -> p h c", h=H),
                    ps_agg[:, 0:HC].rearrange("p (h c) -> p h c", h=H),
                    rcp[:].to_broadcast([P, H, C]), op=AL.mult)
                nc.vector.tensor_tensor(outb[:], outf[:], cb_t[:], op=AL.add)
            else:
                nc.vector.tensor_tensor(
                    outb[:].rearrange("p (h c) -> p h c", h=H),
                    ps_agg[:, 0:HC].rearrange("p (h c) -> p h c", h=H),
                    rcp[:].to_broadcast([P, H, C]), op=AL.mult)
            tp_ps = flp.tile([P, P], bf16, tag="tpps")
            nc.tensor.transpose(tp_ps[:], outb[:], ident_t[:])
            for (j, lo_, hi_) in blk_chunks.get(b, []):
                ci = chunk_ctr[j]
                chunk_ctr[j] += 1
                nc.vector.tensor_reduce(
                    gtmp[:, j, ci:ci + 1],
                    tp_ps[:, lo_:hi_], axis=mybir.AxisListType.X, op=AL.max)

        flp_cm.__exit__(None, None, None)
        agg_cm.__exit__(None, None, None)
        psx_cm.__exit__(None, None, None)
        pst_cm.__exit__(None, None, None)

        # ---------------- pooling + dueling head ----------------
        gacc = fl.tile([P, 8], f32, tag="gacc")
        nc.vector.tensor_reduce(gacc[:], gtmp[:], axis=mybir.AxisListType.X,
                                op=AL.max)
        grelu = fl.tile([P, 8], f32, tag="grelu")
        nc.scalar.activation(grelu[:], gacc[:], AF.Relu)

        with tc.tile_pool(name="mlp", bufs=1, space="PSUM") as mp:
            q1p = mp.tile([MLP_H, 8], f32, tag="q1p")
            nc.tensor.matmul(q1p[:], wq1_t[:], grelu[:], start=True, stop=True)
            q1s = fl.tile([MLP_H, 8], f32, tag="q1s")
            nc.scalar.activation(q1s[:], q1p[:], AF.Relu, bias=bq1_t[:, 0:1])
            v1p = mp.tile([MLP_H, 8], f32, tag="v1p")
            nc.tensor.matmul(v1p[:], wv1_t[:], grelu[:], start=True, stop=True)
            v1s = fl.tile([MLP_H, 8], f32, tag="v1s")
            nc.scalar.activation(v1s[:], v1p[:], AF.Relu, bias=bv1_t[:, 0:1])

            cvp = mp.tile([1, 8], f32, tag="cvp")
            nc.tensor.matmul(cvp[:], wv2_t[:], v1s[:], start=True, stop=False)
            nc.tensor.matmul(cvp[:], wq2nm_t[:], q1s[:], start=False, stop=True)
            corr = fl.tile([1, 8], f32, tag="corr")
            nc.vector.tensor_scalar(corr[:], cvp[:], cadd, None, AL.add)

            q2p = mp.tile([ACT_DIM, 8], f32, tag="q2p")
            nc.tensor.matmul(q2p[:], wq2_t[:], q1s[:], start=True, stop=False)
            nc.tensor.matmul(q2p[:], ones110_t[:], corr[:], start=False, stop=True)
            outsb = fl.tile([ACT_DIM, 8], f32, tag="outsb")
            nc.vector.tensor_scalar(outsb[:], q2p[:], bq2_t[:, 0:1], None, AL.add)
            nc.sync.dma_start(out_q[:], outsb[:])

    nc.compile()
    return nc


def kernel(**inputs):
    if _REPO not in sys.path:
        sys.path.insert(0, _REPO)
    from concourse.bass_utils import run_bass_kernel_spmd

    batch = inputs["batch"]
    assert np.array_equal(batch, ((np.arange(N) * G) // N).astype(batch.dtype))

    prep = _host_prep(inputs)
    (t_lo, t_hi, cum_t, nt_tot, calls, idx_cols,
     idx_all, eslot_all, xrsrc_all, xT, chunks) = prep
    nc = _build(inputs, prep)

    att_flat = np.asarray(inputs["att"], np.float32).reshape(-1)
    shared = dict(
        xT=_bf16(xT),
        wl=_bf16(inputs["Wl"]),
        wr=_bf16(inputs["Wr"]),
        bl4=_bf16(np.tile(inputs["bl"], 4))[None, :],
        br4=_bf16(np.tile(inputs["br"], 4))[None, :],
        ones1=_bf16(np.ones((1, P))),
        iota=_bf16(np.tile(np.arange(P, dtype=np.float32), (P, 1))),
        ident=_bf16(np.eye(P, dtype=np.float32)),
        att4=_bf16(np.tile(att_flat, (P, EGRP))),
        cb=np.tile(np.asarray(inputs["conv_bias"], np.float32), (P, 1)),
        wq1_c=np.asarray(inputs["Wq1"], np.float32),
        wq2_c=np.asarray(inputs["Wq2"], np.float32),
        wv1_c=np.asarray(inputs["Wv1"], np.float32),
        wv2_c=np.asarray(inputs["Wv2"], np.float32),
        wq2nm_c=(-np.asarray(inputs["Wq2"], np.float32).sum(1) / ACT_DIM)[:, None],
        bq1_c=np.asarray(inputs["bq1"], np.float32)[:, None],
        bv1_c=np.asarray(inputs["bv1"], np.float32)[:, None],
        bq2_c=np.asarray(inputs["bq2"], np.float32)[:, None],
        ones110=np.ones((1, ACT_DIM), np.float32),
    )
    in_maps = []
    for k in range(NCORES):
        m = dict(shared)
        m["idx"] = idx_all[k]
        m["eslot"] = _bf16(eslot_all[k])
        m["xrsrc"] = _bf16(xrsrc_all[k])
        in_maps.append(m)

    _cached["nc"] = nc
    _cached["in_maps"] = in_maps
    t0 = time.time()
    res = run_bass_kernel_spmd(nc, in_maps, core_ids=list(range(NCORES)))
    _timing["first_run_s"] = time.time() - t0
    t0 = time.time()
    res = run_bass_kernel_spmd(nc, in_maps, core_ids=list(range(NCORES)))
    _timing["second_run_s"] = time.time() - t0

    out = np.concatenate([res.results[k]["out_q"].T for k in range(NCORES)], axis=0)
    return out.astype(np.float32)
